# revision 1
# baseline (speedup 1.0000x reference)
"""Quantized Linear (8-bit act / 4-bit weight fake-quant) on 8 Trainium2 cores.

Math (per reference):
  xq = rne(x / s_x) * s_x          s_x = max(absmax(x)/127, 1e-8)
  wq = rne(w / s_w) * s_w          s_w = max(absmax(w)/7,   1e-8)
  bq = rne(b / s_b) * s_b          s_b = max(absmax(b)/127, 1e-8)
  out_pre = bq + xq @ wq.T
  out = rne(out_pre / s_o) * s_o   s_o = max(absmax(out_pre)/127, 1e-8)

Device strategy (column-parallel over out_features, 8 cores):
  - Quantized integers Qx in [-127,127] / Qw in [-7,7] are exact in bf16 and
    accumulate exactly in fp32 PSUM, so the matmul runs as an exact integer
    bf16 matmul; scales fold in afterwards: out_pre = (Qx@Qw)*(s_x*s_w) + bq.
  - Round-to-nearest-even via the fp32 magic constant (t + 1.5*2^23 then
    subtract), matching jnp.round exactly.
  - Pipeline per 512-token block: PE transposes raw fp32 x tiles into PSUM;
    ACT evicts with fused scale+magic (y = xT*inv_s + M); DVE finishes the
    round and converts to bf16 (qxT = y - M). No GPSIMD in the data path.
  - Global absmaxes via one tiny AllReduce-max up front (x-slice + w shard)
    and one for out_pre before the final requantization.
  - Each core computes out^T[j_shard, :] = [512, 4096]; host reassembles.
"""

import sys

sys.path.insert(0, "/opt/trn_rl_repo")

import numpy as np

import concourse.bass as bass
import concourse.mybir as mybir
import concourse.tile as tile
from concourse import bacc, bass_isa
from concourse.masks import make_identity

F32 = mybir.dt.float32
BF16 = mybir.dt.bfloat16
AF = mybir.ActivationFunctionType
ALU = mybir.AluOpType
AX = mybir.AxisListType

MAGIC = 12582912.0  # 1.5 * 2**23: fp32 add rounds to nearest-even integer
EPS = 1e-8
INV_QA = float(np.float32(1.0) / np.float32(127.0))
INV_QW = float(np.float32(1.0) / np.float32(7.0))

P = 128


def build(n_cores=8, T=4096, K=4096, J=4096, TB=512):
    JS = J // n_cores
    TS = T // n_cores
    n_kt = K // P
    n_tb = T // TB
    n_m = JS // P
    n_xs = TS // P
    XCH = min(2048, K)
    n_xch = K // XCH
    kpc = XCH // P  # k-tiles per x chunk

    nc = bacc.Bacc(
        "TRN2", target_bir_lowering=False, debug=False, num_devices=n_cores
    )

    x_d = nc.dram_tensor("x", [T, K], F32, kind="ExternalInput")
    w_d = nc.dram_tensor("w_shard", [JS, K], F32, kind="ExternalInput")
    b_d = nc.dram_tensor("b_full", [J], F32, kind="ExternalInput")
    bs_d = nc.dram_tensor("b_shard", [JS], F32, kind="ExternalInput")
    o_d = nc.dram_tensor("outT", [JS, T], F32, kind="ExternalOutput")
    cc1_in = nc.dram_tensor("cc1_in", [1, 2], F32)
    cc1_out = nc.dram_tensor("cc1_out", [1, 2], F32)
    cc2_in = nc.dram_tensor("cc2_in", [1, 1], F32)
    cc2_out = nc.dram_tensor("cc2_out", [1, 1], F32)
    groups = [list(range(n_cores))]

    with tile.TileContext(nc) as tc:
        with (
            tc.tile_pool(name="const", bufs=1) as const,
            tc.tile_pool(name="scal", bufs=1) as scal,
            tc.tile_pool(name="perm", bufs=1) as perm,
            tc.tile_pool(name="stage", bufs=6) as stage,
            tc.tile_pool(name="ypool", bufs=3) as ypool,
            tc.tile_pool(name="rpool", bufs=36) as rpool,
            tc.tile_pool(name="tps", bufs=3, space="PSUM") as tps,
            tc.tile_pool(name="mmps", bufs=5, space="PSUM") as mmps,
        ):
            identf = const.tile([P, P], F32)
            make_identity(nc, identf)
            magic_t = const.tile([P, 1], F32)
            nc.vector.memset(magic_t[:], MAGIC)

            # ---------------- Phase 0: absmax of the exclusive x slice -------
            # x is rotated per core on the host, so rows [0:TS) are this
            # core's exclusive absmax slice; w absmax rides on W-prep loads.
            nx = n_xs * n_xch
            am = scal.tile([P, nx + n_m * n_xch + 1], F32)
            for c in range(n_xs):
                for ch in range(n_xch):
                    t = stage.tile([P, XCH], F32, tag="xf")
                    nc.sync.dma_start(
                        t[:], x_d[c * P : (c + 1) * P, ch * XCH : (ch + 1) * XCH]
                    )
                    nc.vector.tensor_reduce(
                        am[:, c * n_xch + ch : c * n_xch + ch + 1], t[:],
                        axis=AX.X, op=ALU.max, apply_absolute_value=True,
                    )
            bfull = scal.tile([P, J // P], F32)
            nc.gpsimd.dma_start(bfull[:], b_d.rearrange("(p a) -> p a", p=P))
            nc.vector.tensor_reduce(
                am[:, nx + n_m * n_xch :], bfull[:], axis=AX.X, op=ALU.max,
                apply_absolute_value=True,
            )
            for c in range(n_m):
                for ch in range(n_xch):
                    t = stage.tile([P, XCH], F32, tag="xf")
                    nc.scalar.dma_start(
                        t[:], w_d[c * P : (c + 1) * P, ch * XCH : (ch + 1) * XCH]
                    )
                    nc.vector.tensor_reduce(
                        am[:, nx + c * n_xch + ch : nx + c * n_xch + ch + 1],
                        t[:], axis=AX.X, op=ALU.max, apply_absolute_value=True,
                    )

            m2 = scal.tile([P, 2], F32)
            nc.vector.tensor_reduce(m2[:, 0:1], am[:, :nx], axis=AX.X, op=ALU.max)
            nc.vector.tensor_reduce(
                m2[:, 1:2], am[:, nx : nx + n_m * n_xch], axis=AX.X, op=ALU.max
            )
            g2 = scal.tile([P, 2], F32)
            nc.gpsimd.partition_all_reduce(
                g2[:], m2[:], channels=P, reduce_op=bass_isa.ReduceOp.max
            )
            nc.sync.dma_start(cc1_in[:], g2[:1, :])
            nc.gpsimd.collective_compute(
                "AllReduce", ALU.max, replica_groups=groups,
                ins=[cc1_in[:]], outs=[cc1_out[:]],
            )
            gx = scal.tile([P, 2], F32)
            nc.sync.dma_start(gx[:1, :], cc1_out[:])
            bc2 = scal.tile([P, 2], F32)
            nc.gpsimd.partition_broadcast(bc2[:], gx[:1, :], channels=P)

            s_x = scal.tile([P, 1], F32)
            s_w = scal.tile([P, 1], F32)
            s_b = scal.tile([P, 1], F32)
            nc.vector.tensor_scalar(s_x[:], bc2[:, 0:1], INV_QA, EPS, op0=ALU.mult, op1=ALU.max)
            nc.vector.tensor_scalar(s_w[:], bc2[:, 1:2], INV_QW, EPS, op0=ALU.mult, op1=ALU.max)
            bmax = scal.tile([P, 1], F32)
            nc.gpsimd.partition_all_reduce(
                bmax[:], am[:, nx + n_m * n_xch :], channels=P, reduce_op=bass_isa.ReduceOp.max
            )
            nc.vector.tensor_scalar(s_b[:], bmax[:], INV_QA, EPS, op0=ALU.mult, op1=ALU.max)
            inv_sx = scal.tile([P, 1], F32)
            inv_sw = scal.tile([P, 1], F32)
            inv_sb = scal.tile([P, 1], F32)
            nc.vector.reciprocal(inv_sx[:], s_x[:])
            nc.vector.reciprocal(inv_sw[:], s_w[:])
            nc.vector.reciprocal(inv_sb[:], s_b[:])
            s_xw = scal.tile([P, 1], F32)
            nc.vector.tensor_tensor(out=s_xw[:], in0=s_x[:], in1=s_w[:], op=ALU.mult)

            bsh = scal.tile([P, n_m], F32)
            nc.gpsimd.dma_start(bsh[:], bs_d.rearrange("(a p) -> p a", p=P))
            by = scal.tile([P, n_m], F32)
            nc.scalar.activation(by[:], bsh[:], AF.Identity, bias=magic_t[:], scale=inv_sb[:])
            bq = scal.tile([P, n_m], F32)
            nc.vector.tensor_scalar(bq[:], by[:], -MAGIC, s_b[:], op0=ALU.add, op1=ALU.mult)

            # ---------------- W prep: transpose -> quantize -> QwT -----------
            qwT = perm.tile([P, n_kt, JS], BF16)  # [k%128, kt, j]
            for c in range(n_m):
                for ch in range(n_xch):
                    wf = stage.tile([P, XCH], F32, tag="xf")
                    nc.scalar.dma_start(
                        wf[:], w_d[c * P : (c + 1) * P, ch * XCH : (ch + 1) * XCH]
                    )
                    for kp in range(0, kpc, 2):
                        kt = ch * kpc + kp
                        pw = tps.tile([P, 2 * P], F32, tag="tp", name=f"pw_{c}_{kt}")
                        nc.tensor.transpose(
                            pw[:, 0:P], wf[:, kp * P : (kp + 1) * P], identf[:]
                        )
                        nc.tensor.transpose(
                            pw[:, P : 2 * P], wf[:, (kp + 1) * P : (kp + 2) * P], identf[:]
                        )
                        wy = ypool.tile([P, 2 * P], F32, tag="ysb")
                        nc.scalar.activation(
                            wy[:], pw[:], AF.Identity, bias=magic_t[:], scale=inv_sw[:]
                        )
                        wz = ypool.tile([P, 2 * P], BF16, tag="wz")
                        nc.vector.tensor_scalar(wz[:], wy[:], -MAGIC, None, op0=ALU.add)
                        nc.vector.tensor_copy(
                            out=qwT[:, kt, c * P : (c + 1) * P], in_=wz[:, 0:P]
                        )
                        nc.vector.tensor_copy(
                            out=qwT[:, kt + 1, c * P : (c + 1) * P], in_=wz[:, P : 2 * P]
                        )
            # ---------------- Main: transpose x -> quantize -> matmul --------
            opre = perm.tile([P, n_tb * n_m, TB], F32)
            omax = scal.tile([P, n_tb * n_m], F32)
            for tb in range(n_tb):
                qxT_t = []
                for kt in range(n_kt):
                    qxT_t.append(rpool.tile([P, TB], BF16, tag="qxT", name=f"qxT_{tb}_{kt}"))
                for half in range(TB // 256):
                    xf_t = {}
                    for tsh in range(2):
                        row0 = tb * TB + (half * 2 + tsh) * P
                        for ch in range(n_xch):
                            xf = stage.tile([P, XCH], F32, tag="xf")
                            nc.sync.dma_start(
                                xf[:], x_d[row0 : row0 + P, ch * XCH : (ch + 1) * XCH]
                            )
                            xf_t[(tsh, ch)] = xf
                    for kt in range(n_kt):
                        ch, kp = divmod(kt, kpc)
                        pt = tps.tile([P, 2 * P], F32, tag="tp")
                        nc.tensor.transpose(
                            pt[:, 0:P],
                            xf_t[(0, ch)][:, kp * P : (kp + 1) * P],
                            identf[:],
                        )
                        nc.tensor.transpose(
                            pt[:, P : 2 * P],
                            xf_t[(1, ch)][:, kp * P : (kp + 1) * P],
                            identf[:],
                        )
                        ysb = ypool.tile([P, 2 * P], F32, tag="ysb")
                        nc.scalar.activation(
                            ysb[:], pt[:], AF.Identity, bias=magic_t[:], scale=inv_sx[:]
                        )
                        nc.vector.tensor_scalar(
                            qxT_t[kt][:, half * 256 : half * 256 + 256],
                            ysb[:], -MAGIC, None, op0=ALU.add,
                        )
                ps_m = [mmps.tile([P, TB], F32, tag="mm", name=f"psmm_{tb}_{m}") for m in range(n_m)]
                for kt in range(n_kt):
                    for m in range(n_m):
                        nc.tensor.matmul(
                            ps_m[m][:],
                            lhsT=qwT[:, kt, m * P : (m + 1) * P],
                            rhs=qxT_t[kt][:],
                            start=(kt == 0),
                            stop=(kt == n_kt - 1),
                        )
                for m in range(n_m):
                    oc = opre[:, tb * n_m + m, :]
                    nc.scalar.activation(
                        oc, ps_m[m][:], AF.Identity, bias=bq[:, m : m + 1], scale=s_xw[:]
                    )
                    nc.vector.tensor_reduce(
                        omax[:, tb * n_m + m : tb * n_m + m + 1], oc,
                        axis=AX.X, op=ALU.max, apply_absolute_value=True,
                    )

            # ---------------- Tail: global out absmax -> requantize ---------
            om1 = scal.tile([P, 1], F32)
            nc.vector.tensor_reduce(om1[:], omax[:], axis=AX.X, op=ALU.max)
            omr = scal.tile([P, 1], F32)
            nc.gpsimd.partition_all_reduce(
                omr[:], om1[:], channels=P, reduce_op=bass_isa.ReduceOp.max
            )
            nc.sync.dma_start(cc2_in[:], omr[:1, :])
            nc.gpsimd.collective_compute(
                "AllReduce", ALU.max, replica_groups=groups,
                ins=[cc2_in[:]], outs=[cc2_out[:]],
            )
            go = scal.tile([P, 1], F32)
            nc.sync.dma_start(go[:1, :], cc2_out[:])
            bco = scal.tile([P, 1], F32)
            nc.gpsimd.partition_broadcast(bco[:], go[:1, :], channels=P)
            s_o = scal.tile([P, 1], F32)
            nc.vector.tensor_scalar(s_o[:], bco[:], INV_QA, EPS, op0=ALU.mult, op1=ALU.max)
            inv_so = scal.tile([P, 1], F32)
            nc.vector.reciprocal(inv_so[:], s_o[:])

            for tb in range(n_tb):
                for m in range(n_m):
                    oy = ypool.tile([P, TB], F32, tag="oy")
                    res = ypool.tile([P, TB], F32, tag="ores")
                    nc.scalar.activation(
                        oy[:], opre[:, tb * n_m + m, :], AF.Identity,
                        bias=magic_t[:], scale=inv_so[:],
                    )
                    nc.vector.tensor_scalar(res[:], oy[:], -MAGIC, s_o[:], op0=ALU.add, op1=ALU.mult)
                    nc.scalar.dma_start(
                        o_d[m * P : (m + 1) * P, tb * TB : (tb + 1) * TB], res[:]
                    )

    nc.compile()
    return nc


def _run(nc, inputs, n_cores, T, K, J, trace=False):
    from concourse.bass_utils import run_bass_kernel_spmd

    JS, TS = J // n_cores, T // n_cores
    x = np.ascontiguousarray(inputs["x"], dtype=np.float32)
    w = np.ascontiguousarray(inputs["weight"], dtype=np.float32)
    b = np.ascontiguousarray(inputs["b"], dtype=np.float32)
    in_maps = []
    for c in range(n_cores):
        in_maps.append(
            {
                # rotate so core c's exclusive absmax slice is its first block
                "x": np.roll(x, -c * TS, axis=0) if c else x,
                "w_shard": np.ascontiguousarray(w[c * JS : (c + 1) * JS]),
                "b_full": b,
                "b_shard": np.ascontiguousarray(b[c * JS : (c + 1) * JS]),
            }
        )
    res = run_bass_kernel_spmd(nc, in_maps, core_ids=list(range(n_cores)), trace=trace)
    shards = [np.roll(res.results[c]["outT"], c * TS, axis=1) for c in range(n_cores)]
    out = np.ascontiguousarray(np.concatenate(shards, axis=0).T)
    return out, res


_NC_CACHE = {}


def kernel(**inputs) -> np.ndarray:
    n_cores, T, K, J = 8, 4096, 4096, 4096
    key = (n_cores, T, K, J)
    if key not in _NC_CACHE:
        _NC_CACHE[key] = build(n_cores, T, K, J)
    out, _ = _run(_NC_CACHE[key], inputs, n_cores, T, K, J)
    return out



# revision 6
# speedup vs baseline: 1.1554x; 1.1554x over previous
"""Quantized Linear (8-bit act / 4-bit weight fake-quant) on 8 Trainium2 cores.

Math (per reference):
  xq = rne(x / s_x) * s_x          s_x = max(absmax(x)/127, 1e-8)
  wq = rne(w / s_w) * s_w          s_w = max(absmax(w)/7,   1e-8)
  bq = rne(b / s_b) * s_b          s_b = max(absmax(b)/127, 1e-8)
  out_pre = bq + xq @ wq.T
  out = rne(out_pre / s_o) * s_o   s_o = max(absmax(out_pre)/127, 1e-8)

Device strategy (2-way tokens x 4-way out_features, 8 cores):
  - Host pre-transposes x->[K,T] and w->[K,J] so the contraction dim K lands
    on SBUF partitions directly: the PE does ONLY matmuls, no transposes.
  - Quantized integers Qx in [-127,127] / Qw in [-7,7] are exact in bf16 and
    accumulate exactly in fp32 PSUM; scales fold in afterwards:
    out_pre = (Qx@Qw)*(s_x*s_w) + bq.
  - Round-to-nearest-even via the fp32 magic constant (t + 1.5*2^23 then
    subtract), matching jnp.round exactly.
  - Global absmaxes: each core reads an exclusive 1/8 slice of x and of w
    first, reduces, and one tiny AllReduce-max yields s_x/s_w. The x slice
    lands in (and is consumed from) the opre buffer region, so no byte of x
    is read twice. Host rotation makes the exclusive slice the first blocks
    on every core (identical SPMD program).
  - Main loop: per 256-token block, quantize [K,256] tiles (ACT magic-add,
    DVE subtract to bf16) and accumulate 8 PSUM half-banks [128,256] over
    32 k-tiles; evict with fused scale+bias into SBUF-resident opre.
  - Second AllReduce-max over out_pre absmax, then requantize + store.
"""

import sys

sys.path.insert(0, "/opt/trn_rl_repo")

import numpy as np

import concourse.bass as bass
import concourse.mybir as mybir
import concourse.tile as tile
from concourse import bacc, bass_isa

F32 = mybir.dt.float32
BF16 = mybir.dt.bfloat16
AF = mybir.ActivationFunctionType
ALU = mybir.AluOpType
AX = mybir.AxisListType

MAGIC = 12582912.0  # 1.5 * 2**23: fp32 add rounds to nearest-even integer
EPS = 1e-8
INV_QA = float(np.float32(1.0) / np.float32(127.0))
INV_QW = float(np.float32(1.0) / np.float32(7.0))

P = 128
RT, RJ = 2, 4  # token groups x out-feature groups


def build(n_cores=8, T=4096, K=4096, J=4096, TB=256):
    TA = T // RT  # tokens per core
    JB = J // RJ  # out features per core
    n_kt = K // P  # 32 k-tiles
    n_tb = TA // TB  # 8 token blocks
    n_jt = JB // P  # 8 j-tiles
    EXB = (TA // n_cores) // TB  # 2 exclusive x token blocks (512 tokens)
    WEX = JB // RT  # 512 exclusive w columns
    SLOT = n_jt * TB  # opre floats per token block slot (2048)

    nc = bacc.Bacc(
        "TRN2", target_bir_lowering=False, debug=False, num_devices=n_cores
    )

    x_d = nc.dram_tensor("xT_s", [K, TA], F32, kind="ExternalInput")
    w_d = nc.dram_tensor("wT_s", [K, JB], F32, kind="ExternalInput")
    b_d = nc.dram_tensor("b_full", [J], F32, kind="ExternalInput")
    bs_d = nc.dram_tensor("b_shard", [JB], F32, kind="ExternalInput")
    o_d = nc.dram_tensor("outT", [JB, TA], F32, kind="ExternalOutput")
    cc1_in = nc.dram_tensor("cc1_in", [1, 2], F32)
    cc1_out = nc.dram_tensor("cc1_out", [1, 2], F32)
    cc2_in = nc.dram_tensor("cc2_in", [1, 1], F32)
    cc2_out = nc.dram_tensor("cc2_out", [1, 1], F32)
    groups = [list(range(n_cores))]

    with tile.TileContext(nc) as tc:
        with (
            tc.tile_pool(name="const", bufs=1) as const,
            tc.tile_pool(name="scal", bufs=1) as scal,
            tc.tile_pool(name="qwp", bufs=1) as qwp,
            tc.tile_pool(name="qxp", bufs=2) as qxp,
            tc.tile_pool(name="big", bufs=1) as big,
            tc.tile_pool(name="wstage", bufs=2) as wstage,
            tc.tile_pool(name="xstage", bufs=6) as xstage,
            tc.tile_pool(name="ypool", bufs=3) as ypool,
            tc.tile_pool(name="typo", bufs=4) as typo,
            tc.tile_pool(name="mmps", bufs=8, space="PSUM") as mmps,
        ):
            magic_t = const.tile([P, 1], F32)
            nc.vector.memset(magic_t[:], MAGIC)

            # ---------------- Phase 0: exclusive-slice absmaxes -------------
            # Host rotation puts this core's exclusive absmax slice first:
            # x tokens [0:512) and w columns [0:512).
            nx = EXB * n_kt  # 64 x-excl tiles
            am = scal.tile([P, nx + n_kt + 1], F32)

            # opre doubles as raw staging for the exclusive x blocks:
            # raw block i tile kt lives at flat [i*4*SLOT + kt*TB, +TB).
            opre = big.tile([P, n_tb * SLOT], F32)
            for i in range(EXB):
                for kt in range(n_kt):
                    dst = opre[:, i * 4 * SLOT + kt * TB : i * 4 * SLOT + (kt + 1) * TB]
                    nc.sync.dma_start(
                        dst, x_d[kt * P : (kt + 1) * P, i * TB : (i + 1) * TB]
                    )
                    nc.vector.tensor_reduce(
                        am[:, i * n_kt + kt : i * n_kt + kt + 1], dst,
                        axis=AX.X, op=ALU.max, apply_absolute_value=True,
                    )
            for kt in range(n_kt):
                wex = wstage.tile([P, WEX], F32, tag="wf", name=f"wex_{kt}")
                nc.scalar.dma_start(wex[:], w_d[kt * P : (kt + 1) * P, 0:WEX])
                nc.vector.tensor_reduce(
                    am[:, nx + kt : nx + kt + 1], wex[:],
                    axis=AX.X, op=ALU.max, apply_absolute_value=True,
                )
            bfull = scal.tile([P, J // P], F32)
            nc.gpsimd.dma_start(bfull[:], b_d.rearrange("(p a) -> p a", p=P))
            nc.vector.tensor_reduce(
                am[:, nx + n_kt :], bfull[:], axis=AX.X, op=ALU.max,
                apply_absolute_value=True,
            )

            m2 = scal.tile([P, 2], F32)
            nc.vector.tensor_reduce(m2[:, 0:1], am[:, :nx], axis=AX.X, op=ALU.max)
            nc.vector.tensor_reduce(
                m2[:, 1:2], am[:, nx : nx + n_kt], axis=AX.X, op=ALU.max
            )
            g2 = scal.tile([P, 2], F32)
            nc.gpsimd.partition_all_reduce(
                g2[:], m2[:], channels=P, reduce_op=bass_isa.ReduceOp.max
            )
            nc.sync.dma_start(cc1_in[:], g2[:1, :])
            nc.gpsimd.collective_compute(
                "AllReduce", ALU.max, replica_groups=groups,
                ins=[cc1_in[:]], outs=[cc1_out[:]],
            )
            gx = scal.tile([P, 2], F32)
            nc.sync.dma_start(gx[:1, :], cc1_out[:])
            bc2 = scal.tile([P, 2], F32)
            nc.gpsimd.partition_broadcast(bc2[:], gx[:1, :], channels=P)

            s_x = scal.tile([P, 1], F32)
            s_w = scal.tile([P, 1], F32)
            s_b = scal.tile([P, 1], F32)
            nc.vector.tensor_scalar(s_x[:], bc2[:, 0:1], INV_QA, EPS, op0=ALU.mult, op1=ALU.max)
            nc.vector.tensor_scalar(s_w[:], bc2[:, 1:2], INV_QW, EPS, op0=ALU.mult, op1=ALU.max)
            bmax = scal.tile([P, 1], F32)
            nc.gpsimd.partition_all_reduce(
                bmax[:], am[:, nx + n_kt :], channels=P, reduce_op=bass_isa.ReduceOp.max
            )
            nc.vector.tensor_scalar(s_b[:], bmax[:], INV_QA, EPS, op0=ALU.mult, op1=ALU.max)
            inv_sx = scal.tile([P, 1], F32)
            inv_sw = scal.tile([P, 1], F32)
            inv_sb = scal.tile([P, 1], F32)
            nc.vector.reciprocal(inv_sx[:], s_x[:])
            nc.vector.reciprocal(inv_sw[:], s_w[:])
            nc.vector.reciprocal(inv_sb[:], s_b[:])
            s_xw = scal.tile([P, 1], F32)
            nc.vector.tensor_tensor(out=s_xw[:], in0=s_x[:], in1=s_w[:], op=ALU.mult)

            bsh = scal.tile([P, n_jt], F32)
            nc.gpsimd.dma_start(bsh[:], bs_d.rearrange("(a p) -> p a", p=P))
            by = scal.tile([P, n_jt], F32)
            nc.scalar.activation(by[:], bsh[:], AF.Identity, bias=magic_t[:], scale=inv_sb[:])
            bq = scal.tile([P, n_jt], F32)
            nc.vector.tensor_scalar(bq[:], by[:], -MAGIC, s_b[:], op0=ALU.add, op1=ALU.mult)

            # ---------------- Quantize block 0 of x (raw already here) ------
            qwT = qwp.tile([P, n_kt, JB], BF16)
            omax = scal.tile([P, n_tb * n_jt], F32)
            qxb = {}

            def quant_block(p):
                qx = qxp.tile([P, n_kt, TB], BF16, tag="qx", name=f"qx_{p}")
                for kt in range(n_kt):
                    if p < EXB:
                        src = opre[:, p * 4 * SLOT + kt * TB : p * 4 * SLOT + (kt + 1) * TB]
                    else:
                        src = xstage.tile([P, TB], F32, tag="xf")
                        nc.sync.dma_start(
                            src, x_d[kt * P : (kt + 1) * P, p * TB : (p + 1) * TB]
                        )
                    xy = ypool.tile([P, TB], F32, tag="xy")
                    nc.scalar.activation(
                        xy[:], src, AF.Identity, bias=magic_t[:], scale=inv_sx[:]
                    )
                    nc.vector.tensor_scalar(
                        qx[:, kt, :], xy[:], -MAGIC, None, op0=ALU.add
                    )
                return qx

            qxb[0] = quant_block(0)

            # ---------------- W prep: quantize to bf16 [k, j] ---------------
            for kt in range(n_kt):
                wf = wstage.tile([P, JB], F32, tag="wf")
                nc.scalar.dma_start(wf[:], w_d[kt * P : (kt + 1) * P, :])
                wy = ypool.tile([P, JB], F32, tag="wy")
                nc.scalar.activation(
                    wy[:], wf[:], AF.Identity, bias=magic_t[:], scale=inv_sw[:]
                )
                nc.vector.tensor_scalar(
                    qwT[:, kt, :], wy[:], -MAGIC, None, op0=ALU.add
                )

            # ---------------- Main: matmul + evict --------------------------
            for p in range(n_tb):
                if p + 1 < n_tb:
                    qxb[p + 1] = quant_block(p + 1)
                qx = qxb.pop(p)
                ps = [
                    mmps.tile([P, TB], F32, tag="mm", name=f"ps_{p}_{jt}")
                    for jt in range(n_jt)
                ]
                # Two jt-groups of 4 banks: group 0's eviction overlaps
                # group 1's matmul, so the next block never stalls on PSUM.
                for g in range(2):
                    jts = range(g * 4, g * 4 + 4)
                    for kt in range(n_kt):
                        for jt in jts:
                            nc.tensor.matmul(
                                ps[jt][:],
                                lhsT=qwT[:, kt, jt * P : (jt + 1) * P],
                                rhs=qx[:, kt, :],
                                start=(kt == 0),
                                stop=(kt == n_kt - 1),
                            )
                    for jt in jts:
                        oc = opre[:, (p * n_jt + jt) * TB : (p * n_jt + jt + 1) * TB]
                        nc.scalar.activation(
                            oc, ps[jt][:], AF.Identity, bias=bq[:, jt : jt + 1], scale=s_xw[:]
                        )
                        nc.vector.tensor_reduce(
                            omax[:, p * n_jt + jt : p * n_jt + jt + 1], oc,
                            axis=AX.X, op=ALU.max, apply_absolute_value=True,
                        )

            # ---------------- Tail: global out absmax -> requantize ---------
            om1 = scal.tile([P, 1], F32)
            nc.vector.tensor_reduce(om1[:], omax[:], axis=AX.X, op=ALU.max)
            omr = scal.tile([P, 1], F32)
            nc.gpsimd.partition_all_reduce(
                omr[:], om1[:], channels=P, reduce_op=bass_isa.ReduceOp.max
            )
            nc.sync.dma_start(cc2_in[:], omr[:1, :])
            nc.gpsimd.collective_compute(
                "AllReduce", ALU.max, replica_groups=groups,
                ins=[cc2_in[:]], outs=[cc2_out[:]],
            )
            go = scal.tile([P, 1], F32)
            nc.sync.dma_start(go[:1, :], cc2_out[:])
            bco = scal.tile([P, 1], F32)
            nc.gpsimd.partition_broadcast(bco[:], go[:1, :], channels=P)
            s_o = scal.tile([P, 1], F32)
            nc.vector.tensor_scalar(s_o[:], bco[:], INV_QA, EPS, op0=ALU.mult, op1=ALU.max)
            inv_so = scal.tile([P, 1], F32)
            nc.vector.reciprocal(inv_so[:], s_o[:])

            for p in range(n_tb):
                for jt in range(n_jt):
                    src = opre[:, (p * n_jt + jt) * TB : (p * n_jt + jt + 1) * TB]
                    oy = typo.tile([P, TB], F32, tag="oy")
                    res = typo.tile([P, TB], F32, tag="ores")
                    nc.scalar.activation(
                        oy[:], src, AF.Identity, bias=magic_t[:], scale=inv_so[:]
                    )
                    nc.vector.tensor_scalar(
                        res[:], oy[:], -MAGIC, s_o[:], op0=ALU.add, op1=ALU.mult
                    )
                    nc.scalar.dma_start(
                        o_d[jt * P : (jt + 1) * P, p * TB : (p + 1) * TB], res[:]
                    )

    nc.compile()
    return nc


def _run(nc, inputs, n_cores, T, K, J, trace=False):
    from concourse.bass_utils import run_bass_kernel_spmd

    TA, JB = T // RT, J // RJ
    XE, WE = TA // n_cores, JB // RT  # 512, 512 rotation units
    x = np.ascontiguousarray(inputs["x"], dtype=np.float32)
    w = np.ascontiguousarray(inputs["weight"], dtype=np.float32)
    b = np.ascontiguousarray(inputs["b"], dtype=np.float32)
    xT = np.ascontiguousarray(x.T)  # [K, T]
    wT = np.ascontiguousarray(w.T)  # [K, J]
    in_maps = []
    for c in range(n_cores):
        a, bb = divmod(c, RJ)
        xs = xT[:, a * TA : (a + 1) * TA]
        xs = np.ascontiguousarray(np.roll(xs, -bb * XE, axis=1)) if bb else np.ascontiguousarray(xs)
        ws = wT[:, bb * JB : (bb + 1) * JB]
        ws = np.ascontiguousarray(np.roll(ws, -a * WE, axis=1)) if a else np.ascontiguousarray(ws)
        bs = b[bb * JB : (bb + 1) * JB]
        bs = np.ascontiguousarray(np.roll(bs, -a * WE)) if a else np.ascontiguousarray(bs)
        in_maps.append({"xT_s": xs, "wT_s": ws, "b_full": b, "b_shard": bs})
    res = run_bass_kernel_spmd(nc, in_maps, core_ids=list(range(n_cores)), trace=trace)
    out = np.empty((T, J), dtype=np.float32)
    for c in range(n_cores):
        a, bb = divmod(c, RJ)
        ot = res.results[c]["outT"]  # [JB, TA], rotated
        ot = np.roll(ot, bb * XE, axis=1)
        ot = np.roll(ot, a * WE, axis=0)
        out[a * TA : (a + 1) * TA, bb * JB : (bb + 1) * JB] = ot.T
    return out, res


_NC_CACHE = {}


def kernel(**inputs) -> np.ndarray:
    n_cores, T, K, J = 8, 4096, 4096, 4096
    key = (n_cores, T, K, J)
    if key not in _NC_CACHE:
        _NC_CACHE[key] = build(n_cores, T, K, J)
    out, _ = _run(_NC_CACHE[key], inputs, n_cores, T, K, J)
    return out


# revision 7
# speedup vs baseline: 1.3806x; 1.1949x over previous
"""Quantized Linear (8-bit act / 4-bit weight fake-quant) on 8 Trainium2 cores.

Math (per reference):
  xq = rne(x / s_x) * s_x          s_x = max(absmax(x)/127, 1e-8)
  wq = rne(w / s_w) * s_w          s_w = max(absmax(w)/7,   1e-8)
  bq = rne(b / s_b) * s_b          s_b = max(absmax(b)/127, 1e-8)
  out_pre = bq + xq @ wq.T
  out = rne(out_pre / s_o) * s_o   s_o = max(absmax(out_pre)/127, 1e-8)

Device strategy (2-way tokens x 4-way out_features, 8 cores):
  - Host packs per-core inputs k-major so the contraction dim lands on SBUF
    partitions with 8-32KB contiguous DMA lines: x -> [8 blk, 128, 8192]
    (blk-major, [kt,t] flat per partition), w -> [128, 32*1024] ([kt,j] flat).
    The PE does ONLY matmuls; no transposes anywhere.
  - Quantized integers Qx in [-127,127] / Qw in [-7,7] are exact in bf16 and
    accumulate exactly in fp32 PSUM; scales fold in afterwards:
    out_pre = (Qx@Qw)*(s_x*s_w) + bq.
  - Round-to-nearest-even via the fp32 magic constant (t + 1.5*2^23 then
    subtract), matching jnp.round exactly. Quantization runs in [128,2048]
    chunks (ACT magic-add in place, DVE subtract to bf16) to amortize
    per-instruction overheads.
  - Global absmaxes via exclusive slices + one tiny AllReduce-max: each core
    reads 1/8 of x (its first two token blocks, host-rotated; the bytes land
    directly in the opre buffer and are consumed from there) and 1/8 of w
    (a k-tile half, host k-rotation applied to BOTH x and w so the programs
    stay SPMD-identical; contraction order is irrelevant).
  - Per 256-token block: 8 PSUM half-bank accumulators in two jt-groups of 4
    so group 0's eviction overlaps group 1's matmul. Second AllReduce-max
    over out_pre, then requantize + store one 1MiB DMA per block.
"""

import sys

sys.path.insert(0, "/opt/trn_rl_repo")

import numpy as np

import concourse.bass as bass
import concourse.mybir as mybir
import concourse.tile as tile
from concourse import bacc, bass_isa

F32 = mybir.dt.float32
BF16 = mybir.dt.bfloat16
AF = mybir.ActivationFunctionType
ALU = mybir.AluOpType
AX = mybir.AxisListType

MAGIC = 12582912.0  # 1.5 * 2**23: fp32 add rounds to nearest-even integer
EPS = 1e-8
INV_QA = float(np.float32(1.0) / np.float32(127.0))
INV_QW = float(np.float32(1.0) / np.float32(7.0))

P = 128
RT, RJ = 2, 4  # token groups x out-feature groups


def build(n_cores=8, T=4096, K=4096, J=4096, TB=256):
    TA = T // RT  # 2048 tokens per core
    JB = J // RJ  # 1024 out features per core
    n_kt = K // P  # 32 k-tiles
    n_tb = TA // TB  # 8 token blocks
    n_jt = JB // P  # 8 j-tiles
    BLK = n_kt * TB  # 8192 floats per partition per x block
    SLOT = n_jt * TB  # 2048 floats per opre block slot
    WPT = JB  # w floats per partition per k-tile (1024)
    CH = 2048  # elementwise chunk size

    nc = bacc.Bacc(
        "TRN2", target_bir_lowering=False, debug=False, num_devices=n_cores
    )

    x_d = nc.dram_tensor("x_p", [n_tb, P, BLK], F32, kind="ExternalInput")
    w_d = nc.dram_tensor("w_p", [P, n_kt * WPT], F32, kind="ExternalInput")
    b_d = nc.dram_tensor("b_full", [J], F32, kind="ExternalInput")
    bs_d = nc.dram_tensor("b_shard", [JB], F32, kind="ExternalInput")
    o_d = nc.dram_tensor("outT", [JB, TA], F32, kind="ExternalOutput")
    o_r = o_d.rearrange("(jt p) t -> p jt t", p=P)
    cc1_in = nc.dram_tensor("cc1_in", [1, 2], F32)
    cc1_out = nc.dram_tensor("cc1_out", [1, 2], F32)
    cc2_in = nc.dram_tensor("cc2_in", [1, 1], F32)
    cc2_out = nc.dram_tensor("cc2_out", [1, 1], F32)
    groups = [list(range(n_cores))]

    with tile.TileContext(nc) as tc:
        with (
            tc.tile_pool(name="const", bufs=1) as const,
            tc.tile_pool(name="scal", bufs=1) as scal,
            tc.tile_pool(name="qwp", bufs=1) as qwp,
            tc.tile_pool(name="qxp", bufs=2) as qxp,
            tc.tile_pool(name="big", bufs=1) as big,
            tc.tile_pool(name="stage", bufs=3) as stage,
            tc.tile_pool(name="typo", bufs=2) as typo,
            tc.tile_pool(name="mmps", bufs=8, space="PSUM") as mmps,
        ):
            magic_t = const.tile([P, 1], F32)
            nc.vector.memset(magic_t[:], MAGIC)

            # ---------------- Phase 0: exclusive-slice absmaxes -------------
            # x-excl: the (host-rotated) first two token blocks, DMAed
            # straight into their opre staging region. w-excl: the first
            # k-tile half (host k-rotation differs per core).
            nax = 2 * (BLK // 4096)  # 4 x reduce columns
            naw = (n_kt // 2) * WPT // CH  # 8 w reduce columns
            am = scal.tile([P, nax + naw + 1], F32)

            opre = big.tile([P, n_tb * SLOT], F32)
            for i in range(2):
                nc.sync.dma_start(opre[:, i * BLK : (i + 1) * BLK], x_d[i])
                for h in range(BLK // 4096):
                    nc.vector.tensor_reduce(
                        am[:, i * 2 + h : i * 2 + h + 1],
                        opre[:, i * BLK + h * 4096 : i * BLK + (h + 1) * 4096],
                        axis=AX.X, op=ALU.max, apply_absolute_value=True,
                    )
            for e in range(naw):
                wex = stage.tile([P, CH], F32, tag="st", name=f"wex_{e}")
                nc.scalar.dma_start(wex[:], w_d[:, e * CH : (e + 1) * CH])
                nc.vector.tensor_reduce(
                    am[:, nax + e : nax + e + 1], wex[:],
                    axis=AX.X, op=ALU.max, apply_absolute_value=True,
                )
            bfull = scal.tile([P, J // P], F32)
            nc.gpsimd.dma_start(bfull[:], b_d.rearrange("(p a) -> p a", p=P))
            nc.vector.tensor_reduce(
                am[:, nax + naw :], bfull[:], axis=AX.X, op=ALU.max,
                apply_absolute_value=True,
            )

            m2 = scal.tile([P, 2], F32)
            nc.vector.tensor_reduce(m2[:, 0:1], am[:, :nax], axis=AX.X, op=ALU.max)
            nc.vector.tensor_reduce(
                m2[:, 1:2], am[:, nax : nax + naw], axis=AX.X, op=ALU.max
            )
            g2 = scal.tile([P, 2], F32)
            nc.gpsimd.partition_all_reduce(
                g2[:], m2[:], channels=P, reduce_op=bass_isa.ReduceOp.max
            )
            nc.sync.dma_start(cc1_in[:], g2[:1, :])
            nc.gpsimd.collective_compute(
                "AllReduce", ALU.max, replica_groups=groups,
                ins=[cc1_in[:]], outs=[cc1_out[:]],
            )
            gx = scal.tile([P, 2], F32)
            nc.sync.dma_start(gx[:1, :], cc1_out[:])
            bc2 = scal.tile([P, 2], F32)
            nc.gpsimd.partition_broadcast(bc2[:], gx[:1, :], channels=P)

            s_x = scal.tile([P, 1], F32)
            s_w = scal.tile([P, 1], F32)
            s_b = scal.tile([P, 1], F32)
            nc.vector.tensor_scalar(s_x[:], bc2[:, 0:1], INV_QA, EPS, op0=ALU.mult, op1=ALU.max)
            nc.vector.tensor_scalar(s_w[:], bc2[:, 1:2], INV_QW, EPS, op0=ALU.mult, op1=ALU.max)
            bmax = scal.tile([P, 1], F32)
            nc.gpsimd.partition_all_reduce(
                bmax[:], am[:, nax + naw :], channels=P, reduce_op=bass_isa.ReduceOp.max
            )
            nc.vector.tensor_scalar(s_b[:], bmax[:], INV_QA, EPS, op0=ALU.mult, op1=ALU.max)
            inv_sx = scal.tile([P, 1], F32)
            inv_sw = scal.tile([P, 1], F32)
            inv_sb = scal.tile([P, 1], F32)
            nc.vector.reciprocal(inv_sx[:], s_x[:])
            nc.vector.reciprocal(inv_sw[:], s_w[:])
            nc.vector.reciprocal(inv_sb[:], s_b[:])
            s_xw = scal.tile([P, 1], F32)
            nc.vector.tensor_tensor(out=s_xw[:], in0=s_x[:], in1=s_w[:], op=ALU.mult)

            bsh = scal.tile([P, n_jt], F32)
            nc.gpsimd.dma_start(bsh[:], bs_d.rearrange("(a p) -> p a", p=P))
            by = scal.tile([P, n_jt], F32)
            nc.scalar.activation(by[:], bsh[:], AF.Identity, bias=magic_t[:], scale=inv_sb[:])
            bq = scal.tile([P, n_jt], F32)
            nc.vector.tensor_scalar(bq[:], by[:], -MAGIC, s_b[:], op0=ALU.add, op1=ALU.mult)

            # ---------------- Quantization helpers --------------------------
            qwT = qwp.tile([P, n_kt * WPT], BF16)
            omax = scal.tile([P, n_tb * 2], F32)
            qxb = {}

            def quant_block(p):
                qx = qxp.tile([P, BLK], BF16, tag="qx", name=f"qx_{p}")
                for q in range(BLK // CH):
                    if p < 2:
                        reg = opre[:, p * BLK + q * CH : p * BLK + (q + 1) * CH]
                    else:
                        reg = stage.tile([P, CH], F32, tag="st", name=f"xs_{p}_{q}")
                        nc.sync.dma_start(reg, x_d[p, :, q * CH : (q + 1) * CH])
                    nc.scalar.activation(
                        reg, reg, AF.Identity, bias=magic_t[:], scale=inv_sx[:]
                    )
                    nc.vector.tensor_scalar(
                        qx[:, q * CH : (q + 1) * CH], reg, -MAGIC, None, op0=ALU.add
                    )
                return qx

            qxb[0] = quant_block(0)

            # ---------------- W prep: quantize to bf16 ----------------------
            for e in range(n_kt * WPT // CH):
                wf = stage.tile([P, CH], F32, tag="st", name=f"wf_{e}")
                nc.scalar.dma_start(wf[:], w_d[:, e * CH : (e + 1) * CH])
                nc.scalar.activation(
                    wf[:], wf[:], AF.Identity, bias=magic_t[:], scale=inv_sw[:]
                )
                nc.vector.tensor_scalar(
                    qwT[:, e * CH : (e + 1) * CH], wf[:], -MAGIC, None, op0=ALU.add
                )

            # ---------------- Main: matmul + evict --------------------------
            for p in range(n_tb):
                if p + 1 < n_tb:
                    qxb[p + 1] = quant_block(p + 1)
                qx = qxb.pop(p)
                ps = [
                    mmps.tile([P, TB], F32, tag="mm", name=f"ps_{p}_{jt}")
                    for jt in range(n_jt)
                ]
                # Two jt-groups of 4 banks: group 0's eviction overlaps
                # group 1's matmul, so the next block never stalls on PSUM.
                for g in range(2):
                    jts = range(g * 4, g * 4 + 4)
                    for kt in range(n_kt):
                        for jt in jts:
                            nc.tensor.matmul(
                                ps[jt][:],
                                lhsT=qwT[:, kt * WPT + jt * P : kt * WPT + (jt + 1) * P],
                                rhs=qx[:, kt * TB : (kt + 1) * TB],
                                start=(kt == 0),
                                stop=(kt == n_kt - 1),
                            )
                    for jt in jts:
                        oc = opre[:, (p * n_jt + jt) * TB : (p * n_jt + jt + 1) * TB]
                        nc.scalar.activation(
                            oc, ps[jt][:], AF.Identity, bias=bq[:, jt : jt + 1], scale=s_xw[:]
                        )
                    nc.vector.tensor_reduce(
                        omax[:, p * 2 + g : p * 2 + g + 1],
                        opre[:, p * SLOT + g * 4 * TB : p * SLOT + (g + 1) * 4 * TB],
                        axis=AX.X, op=ALU.max, apply_absolute_value=True,
                    )

            # ---------------- Tail: global out absmax -> requantize ---------
            om1 = scal.tile([P, 1], F32)
            nc.vector.tensor_reduce(om1[:], omax[:], axis=AX.X, op=ALU.max)
            omr = scal.tile([P, 1], F32)
            nc.gpsimd.partition_all_reduce(
                omr[:], om1[:], channels=P, reduce_op=bass_isa.ReduceOp.max
            )
            nc.sync.dma_start(cc2_in[:], omr[:1, :])
            nc.gpsimd.collective_compute(
                "AllReduce", ALU.max, replica_groups=groups,
                ins=[cc2_in[:]], outs=[cc2_out[:]],
            )
            go = scal.tile([P, 1], F32)
            nc.sync.dma_start(go[:1, :], cc2_out[:])
            bco = scal.tile([P, 1], F32)
            nc.gpsimd.partition_broadcast(bco[:], go[:1, :], channels=P)
            s_o = scal.tile([P, 1], F32)
            nc.vector.tensor_scalar(s_o[:], bco[:], INV_QA, EPS, op0=ALU.mult, op1=ALU.max)
            inv_so = scal.tile([P, 1], F32)
            nc.vector.reciprocal(inv_so[:], s_o[:])

            for p in range(n_tb):
                src = opre[:, p * SLOT : (p + 1) * SLOT]
                nc.scalar.activation(
                    src, src, AF.Identity, bias=magic_t[:], scale=inv_so[:]
                )
                res = typo.tile([P, SLOT], F32, tag="ores", name=f"res_{p}")
                nc.vector.tensor_scalar(
                    res[:], src, -MAGIC, s_o[:], op0=ALU.add, op1=ALU.mult
                )
                nc.scalar.dma_start(
                    o_r[:, :, p * TB : (p + 1) * TB],
                    res[:].rearrange("p (jt t) -> p jt t", jt=n_jt),
                )

    nc.compile()
    return nc


def _pack_x(xa, a, bb, n_tb=8, TB=256, n_kt=32):
    # xa: [TA, K] token-slice for group a -> [n_tb, 128, n_kt*TB] packed,
    # k-rotated by a (matching w) and token-block-rotated by bb (excl-first).
    t = xa.reshape(n_tb, TB, n_kt, P).transpose(0, 3, 2, 1)  # [tb, p, kt, t]
    t = np.roll(t, -a * (n_kt // 2), axis=2)
    t = np.roll(t, -2 * bb, axis=0)
    return np.ascontiguousarray(t).reshape(n_tb, P, n_kt * TB)


def _pack_w(wb, a, n_kt=32):
    # wb: [JB, K] out-feature slice -> [128, n_kt*JB] packed, k-rotated by a.
    JB = wb.shape[0]
    t = wb.T.reshape(n_kt, P, JB).transpose(1, 0, 2)  # [p, kt, j]
    t = np.roll(t, -a * (n_kt // 2), axis=1)
    return np.ascontiguousarray(t).reshape(P, n_kt * JB)


def _run(nc, inputs, n_cores, T, K, J, trace=False):
    from concourse.bass_utils import run_bass_kernel_spmd

    TA, JB, TB = T // RT, J // RJ, 256
    n_tb = TA // TB
    x = np.ascontiguousarray(inputs["x"], dtype=np.float32)
    w = np.ascontiguousarray(inputs["weight"], dtype=np.float32)
    b = np.ascontiguousarray(inputs["b"], dtype=np.float32)
    in_maps = []
    for c in range(n_cores):
        a, bb = divmod(c, RJ)
        in_maps.append(
            {
                "x_p": _pack_x(x[a * TA : (a + 1) * TA], a, bb, n_tb, TB, K // P),
                "w_p": _pack_w(w[bb * JB : (bb + 1) * JB], a, K // P),
                "b_full": b,
                "b_shard": np.ascontiguousarray(b[bb * JB : (bb + 1) * JB]),
            }
        )
    res = run_bass_kernel_spmd(nc, in_maps, core_ids=list(range(n_cores)), trace=trace)
    out = np.empty((T, J), dtype=np.float32)
    for c in range(n_cores):
        a, bb = divmod(c, RJ)
        ot = res.results[c]["outT"]  # [JB, TA], token blocks rotated by bb
        ot = ot.reshape(JB, n_tb, TB)
        ot = np.roll(ot, 2 * bb, axis=1).reshape(JB, TA)
        out[a * TA : (a + 1) * TA, bb * JB : (bb + 1) * JB] = ot.T
    return out, res


_NC_CACHE = {}


def kernel(**inputs) -> np.ndarray:
    n_cores, T, K, J = 8, 4096, 4096, 4096
    key = (n_cores, T, K, J)
    if key not in _NC_CACHE:
        _NC_CACHE[key] = build(n_cores, T, K, J)
    out, _ = _run(_NC_CACHE[key], inputs, n_cores, T, K, J)
    return out


# revision 15
# speedup vs baseline: 1.4338x; 1.0386x over previous
"""Quantized Linear (8-bit act / 4-bit weight fake-quant) on 8 Trainium2 cores.

Math (per reference):
  xq = rne(x / s_x) * s_x          s_x = max(absmax(x)/127, 1e-8)
  wq = rne(w / s_w) * s_w          s_w = max(absmax(w)/7,   1e-8)
  bq = rne(b / s_b) * s_b          s_b = max(absmax(b)/127, 1e-8)
  out_pre = bq + xq @ wq.T
  out = rne(out_pre / s_o) * s_o   s_o = max(absmax(out_pre)/127, 1e-8)

Device strategy (2-way tokens x 4-way out_features, 8 cores):
  - Host packs per-core inputs k-major so the contraction dim lands on SBUF
    partitions with 8-32KB contiguous DMA lines: x -> [8 blk, 128, 8192]
    (blk-major, [kt,t] flat per partition), w -> [128, 32*1024] ([kt,j] flat).
    The PE does ONLY matmuls; no transposes anywhere.
  - Quantized integers Qx in [-127,127] / Qw in [-7,7] are exact in bf16 and
    accumulate exactly in fp32 PSUM; scales fold in afterwards:
    out_pre = (Qx@Qw)*(s_x*s_w) + bq.
  - Round-to-nearest-even via the fp32 magic constant (t + 1.5*2^23 then
    subtract). Quantization runs in [128,2048] chunks (ACT magic-add in
    place, DVE subtract to bf16) to amortize per-instruction overheads.
  - Global absmaxes via exclusive slices + one tiny AllReduce-max: each core
    reads 1/8 of x (its first two token blocks, host-rotated; bytes land in
    the opre buffer and are consumed from there) and 1/8 of w (a k-tile
    half; the host k-rotates BOTH x and w per core, so programs stay
    SPMD-identical and contraction order is irrelevant). DVE absmax reduces
    are issued in expected data-arrival order to avoid head-of-line blocks;
    W main loads issue from the (idle) PE queue so no ring stalls them.
  - Per 256-token block: 8 PSUM half-bank accumulators; block 0 runs one
    8-bank group (it chases the W DMA stream anyway), later blocks run two
    jt-groups of 4 so eviction overlaps the other group's matmul. Second
    AllReduce-max over out_pre, then requantize + store per half block.
"""

import sys

sys.path.insert(0, "/opt/trn_rl_repo")

import numpy as np

import concourse.bass as bass
import concourse.mybir as mybir
import concourse.tile as tile
from concourse import bacc, bass_isa

F32 = mybir.dt.float32
BF16 = mybir.dt.bfloat16
AF = mybir.ActivationFunctionType
ALU = mybir.AluOpType
AX = mybir.AxisListType

MAGIC = 12582912.0  # 1.5 * 2**23: fp32 add rounds to nearest-even integer
EPS = 1e-8
INV_QA = float(np.float32(1.0) / np.float32(127.0))
INV_QW = float(np.float32(1.0) / np.float32(7.0))

P = 128
RT, RJ = 2, 4  # token groups x out-feature groups


def build(n_cores=8, T=4096, K=4096, J=4096, TB=256):
    TA = T // RT  # 2048 tokens per core
    JB = J // RJ  # 1024 out features per core
    n_kt = K // P  # 32 k-tiles
    n_tb = TA // TB  # 8 token blocks
    n_jt = JB // P  # 8 j-tiles
    BLK = n_kt * TB  # 8192 floats per partition per x block
    SLOT = n_jt * TB  # 2048 floats per opre block slot
    WPT = JB  # w floats per partition per k-tile (1024)
    CH = 2048  # elementwise chunk size
    NWC = n_kt * WPT // CH  # 16 w chunks
    NXC = BLK // CH  # 4 x chunks per block

    nc = bacc.Bacc(
        "TRN2", target_bir_lowering=False, debug=False, num_devices=n_cores
    )

    x_d = nc.dram_tensor("x_p", [n_tb, P, BLK], F32, kind="ExternalInput")
    w_d = nc.dram_tensor("w_p", [P, n_kt * WPT], F32, kind="ExternalInput")
    b_d = nc.dram_tensor("b_full", [J], F32, kind="ExternalInput")
    bs_d = nc.dram_tensor("b_shard", [JB], F32, kind="ExternalInput")
    o_d = nc.dram_tensor("outT", [JB, TA], F32, kind="ExternalOutput")
    o_r = o_d.rearrange("(jt p) t -> p jt t", p=P)
    cc1_in = nc.dram_tensor("cc1_in", [1, 2], F32)
    cc1_out = nc.dram_tensor("cc1_out", [1, 2], F32)
    cc2_in = nc.dram_tensor("cc2_in", [1, 1], F32)
    cc2_out = nc.dram_tensor("cc2_out", [1, 1], F32)
    groups = [list(range(n_cores))]

    with tile.TileContext(nc) as tc:
        with (
            tc.tile_pool(name="const", bufs=1) as const,
            tc.tile_pool(name="scal", bufs=1) as scal,
            tc.tile_pool(name="qwp", bufs=1) as qwp,
            tc.tile_pool(name="qxp", bufs=2) as qxp,
            tc.tile_pool(name="big", bufs=1) as big,
            tc.tile_pool(name="stage", bufs=4) as stage,
            tc.tile_pool(name="typo", bufs=2) as typo,
            tc.tile_pool(name="mmps", bufs=8, space="PSUM") as mmps,
        ):
            magic_t = const.tile([P, 1], F32)
            nc.vector.memset(magic_t[:], MAGIC)
            omax = scal.tile([P, n_tb * 2], F32)
            nc.vector.memset(omax[:], 0.0)

            # ---------------- Phase 0: exclusive-slice absmaxes -------------
            nax = 2 * (BLK // 4096)  # 4 x reduce columns
            naw = (n_kt // 2) * WPT // CH  # 8 w reduce columns
            am = scal.tile([P, nax + naw + 1], F32)

            opre = big.tile([P, n_tb * SLOT], F32)
            # issue all exclusive-slice DMAs first ...
            for i in range(2):
                nc.sync.dma_start(opre[:, i * BLK : (i + 1) * BLK], x_d[i])
            wex = []
            for e in range(naw):
                t = stage.tile([P, CH], F32, tag="st", name=f"wex_{e}")
                nc.scalar.dma_start(t[:], w_d[:, e * CH : (e + 1) * CH])
                wex.append(t)
            # First 4 W main chunks fit in free stage buffers: stream them
            # during the collective window (fabric-queued behind the
            # exclusive reads, which gate the collective).
            wf = []
            for e in range(4):
                t = stage.tile([P, CH], F32, tag="st", name=f"wf_{e}")
                nc.scalar.dma_start(t[:], w_d[:, e * CH : (e + 1) * CH])
                wf.append(t)
            bfull = scal.tile([P, J // P], F32)
            nc.gpsimd.dma_start(bfull[:], b_d.rearrange("(p a) -> p a", p=P))
            bsh = scal.tile([P, n_jt], F32)
            nc.gpsimd.dma_start(bsh[:], bs_d.rearrange("(a p) -> p a", p=P))
            # ... then DVE reduces in expected arrival order (w pieces land
            # every ~3.5us; x block i completes at ~30/45us).
            def wred(e):
                nc.vector.tensor_reduce(
                    am[:, nax + e : nax + e + 1], wex[e][:],
                    axis=AX.X, op=ALU.max, apply_absolute_value=True,
                )

            def xred(i, h):
                nc.vector.tensor_reduce(
                    am[:, i * 2 + h : i * 2 + h + 1],
                    opre[:, i * BLK + h * 4096 : i * BLK + (h + 1) * 4096],
                    axis=AX.X, op=ALU.max, apply_absolute_value=True,
                )

            for e in range(4):
                wred(e)
            xred(0, 0)
            xred(0, 1)
            for e in range(4, 7):
                wred(e)
            xred(1, 0)
            xred(1, 1)
            wred(7)
            nc.vector.tensor_reduce(
                am[:, nax + naw :], bfull[:], axis=AX.X, op=ALU.max,
                apply_absolute_value=True,
            )

            m2 = scal.tile([P, 2], F32)
            nc.vector.tensor_reduce(m2[:, 0:1], am[:, :nax], axis=AX.X, op=ALU.max)
            nc.vector.tensor_reduce(
                m2[:, 1:2], am[:, nax : nax + naw], axis=AX.X, op=ALU.max
            )
            g2 = scal.tile([P, 2], F32)
            nc.gpsimd.partition_all_reduce(
                g2[:], m2[:], channels=P, reduce_op=bass_isa.ReduceOp.max
            )
            nc.sync.dma_start(cc1_in[:], g2[:1, :])
            nc.gpsimd.collective_compute(
                "AllReduce", ALU.max, replica_groups=groups,
                ins=[cc1_in[:]], outs=[cc1_out[:]],
            )
            gx = scal.tile([P, 2], F32)
            nc.sync.dma_start(gx[:1, :], cc1_out[:])
            # Stream the remaining W chunks as soon as the collective is done
            # (their stage buffers only free up once quantization consumes
            # the early chunks, so these must not block any earlier ring op).
            for e in range(4, NWC):
                t = stage.tile([P, CH], F32, tag="st", name=f"wf_{e}")
                nc.sync.dma_start(t[:], w_d[:, e * CH : (e + 1) * CH])
                wf.append(t)
            bc2 = scal.tile([P, 2], F32)
            nc.gpsimd.partition_broadcast(bc2[:], gx[:1, :], channels=P)

            s_x = scal.tile([P, 1], F32)
            s_w = scal.tile([P, 1], F32)
            nc.vector.tensor_scalar(s_x[:], bc2[:, 0:1], INV_QA, EPS, op0=ALU.mult, op1=ALU.max)
            inv_sx = scal.tile([P, 1], F32)
            nc.vector.reciprocal(inv_sx[:], s_x[:])
            nc.vector.tensor_scalar(s_w[:], bc2[:, 1:2], INV_QW, EPS, op0=ALU.mult, op1=ALU.max)
            inv_sw = scal.tile([P, 1], F32)
            nc.vector.reciprocal(inv_sw[:], s_w[:])

            # ---------------- Quantization helpers --------------------------
            qwT = qwp.tile([P, n_kt * WPT], BF16)
            qxb = {}

            def quant_chunk(p, q, qx):
                if p < 2:
                    reg = opre[:, p * BLK + q * CH : p * BLK + (q + 1) * CH]
                else:
                    reg = stage.tile([P, CH], F32, tag="st", name=f"xs_{p}_{q}")
                    nc.sync.dma_start(reg, x_d[p, :, q * CH : (q + 1) * CH])
                nc.scalar.activation(
                    reg, reg, AF.Identity, bias=magic_t[:], scale=inv_sx[:]
                )
                nc.vector.tensor_scalar(
                    qx[:, q * CH : (q + 1) * CH], reg, -MAGIC, None, op0=ALU.add
                )

            def quant_block(p):
                qx = qxp.tile([P, BLK], BF16, tag="qx", name=f"qx_{p}")
                for q in range(NXC):
                    quant_chunk(p, q, qx)
                return qx

            def wquant_chunk(e):
                nc.scalar.activation(
                    wf[e][:], wf[e][:], AF.Identity, bias=magic_t[:], scale=inv_sw[:]
                )
                nc.vector.tensor_scalar(
                    qwT[:, e * CH : (e + 1) * CH], wf[e][:], -MAGIC, None, op0=ALU.add
                )

            # Interleave: block 0+1 x-chunks woven between w chunks so the
            # first matmuls (and the first eviction) are never queue-blocked.
            qxb[0] = qxp.tile([P, BLK], BF16, tag="qx", name="qx_0")
            qxb[1] = qxp.tile([P, BLK], BF16, tag="qx", name="qx_1")
            for e in range(NWC):
                if e < NXC:
                    quant_chunk(0, e, qxb[0])
                wquant_chunk(e)
                if NWC - e <= NXC:
                    quant_chunk(1, e - (NWC - NXC), qxb[1])

            s_xw = scal.tile([P, 1], F32)
            nc.vector.tensor_tensor(out=s_xw[:], in0=s_x[:], in1=s_w[:], op=ALU.mult)
            s_b = scal.tile([P, 1], F32)
            bmax = scal.tile([P, 1], F32)
            nc.gpsimd.partition_all_reduce(
                bmax[:], am[:, nax + naw :], channels=P, reduce_op=bass_isa.ReduceOp.max
            )
            nc.vector.tensor_scalar(s_b[:], bmax[:], INV_QA, EPS, op0=ALU.mult, op1=ALU.max)
            inv_sb = scal.tile([P, 1], F32)
            nc.vector.reciprocal(inv_sb[:], s_b[:])
            by = scal.tile([P, n_jt], F32)
            nc.scalar.activation(by[:], bsh[:], AF.Identity, bias=magic_t[:], scale=inv_sb[:])
            bq = scal.tile([P, n_jt], F32)
            nc.vector.tensor_scalar(bq[:], by[:], -MAGIC, s_b[:], op0=ALU.add, op1=ALU.mult)

            # ---------------- Main: matmul + evict --------------------------
            for p in range(n_tb):
                if p + 2 < n_tb:
                    qxb[p + 2] = quant_block(p + 2)
                qx = qxb.pop(p)
                ps = [
                    mmps.tile([P, TB], F32, tag="mm", name=f"ps_{p}_{jt}")
                    for jt in range(n_jt)
                ]
                grps = [range(n_jt)] if p == 0 else [range(4), range(4, n_jt)]
                for g, jts in enumerate(grps):
                    for kt in range(n_kt):
                        for jt in jts:
                            nc.tensor.matmul(
                                ps[jt][:],
                                lhsT=qwT[:, kt * WPT + jt * P : kt * WPT + (jt + 1) * P],
                                rhs=qx[:, kt * TB : (kt + 1) * TB],
                                start=(kt == 0),
                                stop=(kt == n_kt - 1),
                            )
                    for jt in jts:
                        oc = opre[:, (p * n_jt + jt) * TB : (p * n_jt + jt + 1) * TB]
                        nc.scalar.activation(
                            oc, ps[jt][:], AF.Identity, bias=bq[:, jt : jt + 1], scale=s_xw[:]
                        )
                    lo = p * SLOT + jts[0] * TB
                    nc.vector.tensor_reduce(
                        omax[:, p * 2 + g : p * 2 + g + 1],
                        opre[:, lo : lo + len(jts) * TB],
                        axis=AX.X, op=ALU.max, apply_absolute_value=True,
                    )

            # ---------------- Tail: global out absmax -> requantize ---------
            om1 = scal.tile([P, 1], F32)
            nc.vector.tensor_reduce(om1[:], omax[:], axis=AX.X, op=ALU.max)
            omr = scal.tile([P, 1], F32)
            nc.gpsimd.partition_all_reduce(
                omr[:], om1[:], channels=P, reduce_op=bass_isa.ReduceOp.max
            )
            nc.sync.dma_start(cc2_in[:], omr[:1, :])
            nc.gpsimd.collective_compute(
                "AllReduce", ALU.max, replica_groups=groups,
                ins=[cc2_in[:]], outs=[cc2_out[:]],
            )
            go = scal.tile([P, 1], F32)
            nc.sync.dma_start(go[:1, :], cc2_out[:])
            bco = scal.tile([P, 1], F32)
            nc.gpsimd.partition_broadcast(bco[:], go[:1, :], channels=P)
            s_o = scal.tile([P, 1], F32)
            nc.vector.tensor_scalar(s_o[:], bco[:], INV_QA, EPS, op0=ALU.mult, op1=ALU.max)
            inv_so = scal.tile([P, 1], F32)
            nc.vector.reciprocal(inv_so[:], s_o[:])

            HS = SLOT // 2  # half-slot (4 jt) granularity for the tail
            for p in range(n_tb):
                for h in range(2):
                    src = opre[:, p * SLOT + h * HS : p * SLOT + (h + 1) * HS]
                    nc.scalar.activation(
                        src, src, AF.Identity, bias=magic_t[:], scale=inv_so[:]
                    )
                    res = typo.tile([P, HS], F32, tag="ores", name=f"res_{p}_{h}")
                    nc.vector.tensor_scalar(
                        res[:], src, -MAGIC, s_o[:], op0=ALU.add, op1=ALU.mult
                    )
                    nc.scalar.dma_start(
                        o_r[:, h * 4 : (h + 1) * 4, p * TB : (p + 1) * TB],
                        res[:].rearrange("p (jt t) -> p jt t", jt=4),
                    )

    nc.compile()
    return nc


def _pack_x(xa, a, bb, n_tb=8, TB=256, n_kt=32):
    # xa: [TA, K] token-slice for group a -> [n_tb, 128, n_kt*TB] packed,
    # k-rotated by a (matching w) and token-block-rotated by bb (excl-first).
    t = xa.reshape(n_tb, TB, n_kt, P).transpose(0, 3, 2, 1)  # [tb, p, kt, t]
    t = np.roll(t, -a * (n_kt // 2), axis=2)
    t = np.roll(t, -2 * bb, axis=0)
    return np.ascontiguousarray(t).reshape(n_tb, P, n_kt * TB)


def _pack_w(wb, a, n_kt=32):
    # wb: [JB, K] out-feature slice -> [128, n_kt*JB] packed, k-rotated by a.
    JB = wb.shape[0]
    t = wb.T.reshape(n_kt, P, JB).transpose(1, 0, 2)  # [p, kt, j]
    t = np.roll(t, -a * (n_kt // 2), axis=1)
    return np.ascontiguousarray(t).reshape(P, n_kt * JB)


def _run(nc, inputs, n_cores, T, K, J, trace=False):
    from concourse.bass_utils import run_bass_kernel_spmd

    TA, JB, TB = T // RT, J // RJ, 256
    n_tb = TA // TB
    x = np.ascontiguousarray(inputs["x"], dtype=np.float32)
    w = np.ascontiguousarray(inputs["weight"], dtype=np.float32)
    b = np.ascontiguousarray(inputs["b"], dtype=np.float32)
    in_maps = []
    for c in range(n_cores):
        a, bb = divmod(c, RJ)
        in_maps.append(
            {
                "x_p": _pack_x(x[a * TA : (a + 1) * TA], a, bb, n_tb, TB, K // P),
                "w_p": _pack_w(w[bb * JB : (bb + 1) * JB], a, K // P),
                "b_full": b,
                "b_shard": np.ascontiguousarray(b[bb * JB : (bb + 1) * JB]),
            }
        )
    res = run_bass_kernel_spmd(nc, in_maps, core_ids=list(range(n_cores)), trace=trace)
    out = np.empty((T, J), dtype=np.float32)
    for c in range(n_cores):
        a, bb = divmod(c, RJ)
        ot = res.results[c]["outT"]  # [JB, TA], token blocks rotated by bb
        ot = ot.reshape(JB, n_tb, TB)
        ot = np.roll(ot, 2 * bb, axis=1).reshape(JB, TA)
        out[a * TA : (a + 1) * TA, bb * JB : (bb + 1) * JB] = ot.T
    return out, res


_NC_CACHE = {}


def kernel(**inputs) -> np.ndarray:
    n_cores, T, K, J = 8, 4096, 4096, 4096
    key = (n_cores, T, K, J)
    if key not in _NC_CACHE:
        _NC_CACHE[key] = build(n_cores, T, K, J)
    out, _ = _run(_NC_CACHE[key], inputs, n_cores, T, K, J)
    return out


# revision 19
# speedup vs baseline: 1.4795x; 1.0319x over previous
"""Quantized Linear (8-bit act / 4-bit weight fake-quant) on 8 Trainium2 cores.

Math (per reference):
  xq = rne(x / s_x) * s_x          s_x = max(absmax(x)/127, 1e-8)
  wq = rne(w / s_w) * s_w          s_w = max(absmax(w)/7,   1e-8)
  bq = rne(b / s_b) * s_b          s_b = max(absmax(b)/127, 1e-8)
  out_pre = bq + xq @ wq.T
  out = rne(out_pre / s_o) * s_o   s_o = max(absmax(out_pre)/127, 1e-8)

Device strategy (2-way tokens x 4-way out_features, 8 cores):
  - Host packs per-core inputs k-major so the contraction dim lands on SBUF
    partitions with 8-32KB contiguous DMA lines: x -> [8 blk, 128, 8192]
    (blk-major, [kt,t] flat per partition), w -> [128, 32*1024] ([kt,j] flat).
    The PE does ONLY matmuls; no transposes anywhere.
  - Quantized integers Qx in [-127,127] / Qw in [-7,7] are exact in bf16 and
    accumulate exactly in fp32 PSUM; scales fold in afterwards:
    out_pre = (Qx@Qw)*(s_x*s_w) + bq.
  - Round-to-nearest-even via the fp32 magic constant (t + 1.5*2^23 then
    subtract). Quantization runs in [128,2048] chunks (ACT magic-add in
    place, DVE subtract to bf16) to amortize per-instruction overheads.
  - Global absmaxes via exclusive slices + one tiny AllReduce-max: each core
    reads 1/8 of x (its first two token blocks, host-rotated; bytes land in
    the opre buffer and are consumed from there) and 1/8 of w (a k-tile
    half; the host k-rotates BOTH x and w per core, so programs stay
    SPMD-identical and contraction order is irrelevant). DVE absmax reduces
    are issued in expected data-arrival order to avoid head-of-line blocks;
    W main loads issue from the (idle) PE queue so no ring stalls them.
  - Per 256-token block: 8 PSUM half-bank accumulators; block 0 runs one
    8-bank group (it chases the W DMA stream anyway), later blocks run two
    jt-groups of 4 so eviction overlaps the other group's matmul. Second
    AllReduce-max over out_pre, then requantize + store per half block.
"""

import sys

sys.path.insert(0, "/opt/trn_rl_repo")

import numpy as np

import concourse.bass as bass
import concourse.mybir as mybir
import concourse.tile as tile
from concourse import bacc, bass_isa

F32 = mybir.dt.float32
BF16 = mybir.dt.bfloat16
AF = mybir.ActivationFunctionType
ALU = mybir.AluOpType
AX = mybir.AxisListType

MAGIC = 12582912.0  # 1.5 * 2**23: fp32 add rounds to nearest-even integer
EPS = 1e-8
INV_QA = float(np.float32(1.0) / np.float32(127.0))
INV_QW = float(np.float32(1.0) / np.float32(7.0))

P = 128
RT, RJ = 2, 4  # token groups x out-feature groups


def build(n_cores=8, T=4096, K=4096, J=4096, TB=256):
    TA = T // RT  # 2048 tokens per core
    JB = J // RJ  # 1024 out features per core
    n_kt = K // P  # 32 k-tiles
    n_tb = TA // TB  # 8 token blocks
    n_jt = JB // P  # 8 j-tiles
    BLK = n_kt * TB  # 8192 floats per partition per x block
    SLOT = n_jt * TB  # 2048 floats per opre block slot
    WPT = JB  # w floats per partition per k-tile (1024)
    CH = 2048  # elementwise chunk size
    NWC = n_kt * WPT // CH  # 16 w chunks
    NXC = BLK // CH  # 4 x chunks per block

    nc = bacc.Bacc(
        "TRN2", target_bir_lowering=False, debug=False, num_devices=n_cores
    )

    x_d = nc.dram_tensor("x_p", [n_tb, P, BLK], F32, kind="ExternalInput")
    w_d = nc.dram_tensor("w_p", [P, n_kt * WPT], F32, kind="ExternalInput")
    b_d = nc.dram_tensor("b_full", [P, J // P], F32, kind="ExternalInput")
    bs_d = nc.dram_tensor("b_shard", [P, JB // P], F32, kind="ExternalInput")
    o_d = nc.dram_tensor("outT", [JB, TA], F32, kind="ExternalOutput")
    o_r = o_d.rearrange("(jt p) t -> p jt t", p=P)
    cc1_in = nc.dram_tensor("cc1_in", [1, 2], F32)
    cc1_out = nc.dram_tensor("cc1_out", [1, 2], F32)
    cc2_in = nc.dram_tensor("cc2_in", [1, 1], F32)
    cc2_out = nc.dram_tensor("cc2_out", [1, 1], F32)
    groups = [list(range(n_cores))]

    with tile.TileContext(nc) as tc:
        with (
            tc.tile_pool(name="const", bufs=1) as const,
            tc.tile_pool(name="scal", bufs=1) as scal,
            tc.tile_pool(name="qwp", bufs=1) as qwp,
            tc.tile_pool(name="qxp", bufs=2) as qxp,
            tc.tile_pool(name="big", bufs=1) as big,
            tc.tile_pool(name="stage", bufs=4) as stage,
            tc.tile_pool(name="typo", bufs=2) as typo,
            tc.tile_pool(name="mmps", bufs=8, space="PSUM") as mmps,
        ):
            magic_t = const.tile([P, 1], F32)
            nc.vector.memset(magic_t[:], MAGIC)
            omax = scal.tile([P, n_tb * 2], F32)
            nc.vector.memset(omax[:], 0.0)

            # ---------------- Phase 0: exclusive-slice absmaxes -------------
            nax = 2 * (BLK // 4096)  # 4 x reduce columns
            naw = (n_kt // 2) * WPT // CH  # 8 w reduce columns
            am = scal.tile([P, nax + naw + 1], F32)

            opre = big.tile([P, n_tb * SLOT], F32)
            # issue all exclusive-slice DMAs first ...
            for i in range(2):
                nc.sync.dma_start(opre[:, i * BLK : (i + 1) * BLK], x_d[i])
            wex = []
            for e in range(naw):
                t = stage.tile([P, CH], F32, tag="st", name=f"wex_{e}")
                nc.scalar.dma_start(t[:], w_d[:, e * CH : (e + 1) * CH])
                wex.append(t)
            bfull = scal.tile([P, J // P], F32)
            nc.gpsimd.dma_start(bfull[:], b_d[:, :])
            bsh = scal.tile([P, n_jt], F32)
            nc.gpsimd.dma_start(bsh[:], bs_d[:, :])
            # ... then DVE reduces in expected arrival order (w pieces land
            # every ~3.5us; x block i completes at ~30/45us).
            def wred(e):
                nc.vector.tensor_reduce(
                    am[:, nax + e : nax + e + 1], wex[e][:],
                    axis=AX.X, op=ALU.max, apply_absolute_value=True,
                )

            def xred(i, h):
                nc.vector.tensor_reduce(
                    am[:, i * 2 + h : i * 2 + h + 1],
                    opre[:, i * BLK + h * 4096 : i * BLK + (h + 1) * 4096],
                    axis=AX.X, op=ALU.max, apply_absolute_value=True,
                )

            for e in range(4):
                wred(e)
            xred(0, 0)
            xred(0, 1)
            for e in range(4, 7):
                wred(e)
            xred(1, 0)
            xred(1, 1)
            wred(7)
            nc.vector.tensor_reduce(
                am[:, nax + naw :], bfull[:], axis=AX.X, op=ALU.max,
                apply_absolute_value=True,
            )

            m2 = scal.tile([P, 2], F32)
            nc.vector.tensor_reduce(m2[:, 0:1], am[:, :nax], axis=AX.X, op=ALU.max)
            nc.vector.tensor_reduce(
                m2[:, 1:2], am[:, nax : nax + naw], axis=AX.X, op=ALU.max
            )
            g2 = scal.tile([P, 2], F32)
            nc.gpsimd.partition_all_reduce(
                g2[:], m2[:], channels=P, reduce_op=bass_isa.ReduceOp.max
            )
            nc.sync.dma_start(cc1_in[:], g2[:1, :])
            # First 4 W main chunks fit in free stage buffers: stream them
            # during the collective wait, when the DMA fabric is idle (after
            # the exclusive reads that gate the collective are done).
            wf = []
            for e in range(4):
                t = stage.tile([P, CH], F32, tag="st", name=f"wf_{e}")
                nc.sync.dma_start(t[:], w_d[:, e * CH : (e + 1) * CH])
                wf.append(t)
            nc.gpsimd.collective_compute(
                "AllReduce", ALU.max, replica_groups=groups,
                ins=[cc1_in[:]], outs=[cc1_out[:]],
            )
            gx = scal.tile([P, 2], F32)
            nc.sync.dma_start(gx[:1, :], cc1_out[:])
            # Stream the remaining W chunks as soon as the collective is done
            # (their stage buffers only free up once quantization consumes
            # the early chunks, so these must not block any earlier ring op).
            for e in range(4, NWC):
                t = stage.tile([P, CH], F32, tag="st", name=f"wf_{e}")
                nc.sync.dma_start(t[:], w_d[:, e * CH : (e + 1) * CH])
                wf.append(t)
            bc2 = scal.tile([P, 2], F32)
            nc.gpsimd.partition_broadcast(bc2[:], gx[:1, :], channels=P)

            s_x = scal.tile([P, 1], F32)
            s_w = scal.tile([P, 1], F32)
            nc.vector.tensor_scalar(s_x[:], bc2[:, 0:1], INV_QA, EPS, op0=ALU.mult, op1=ALU.max)
            inv_sx = scal.tile([P, 1], F32)
            nc.vector.reciprocal(inv_sx[:], s_x[:])
            nc.vector.tensor_scalar(s_w[:], bc2[:, 1:2], INV_QW, EPS, op0=ALU.mult, op1=ALU.max)
            inv_sw = scal.tile([P, 1], F32)
            nc.vector.reciprocal(inv_sw[:], s_w[:])

            # ---------------- Quantization helpers --------------------------
            qwT = qwp.tile([P, n_kt * WPT], BF16)
            qxb = {}

            def quant_chunk(p, q, qx):
                if p < 2:
                    reg = opre[:, p * BLK + q * CH : p * BLK + (q + 1) * CH]
                else:
                    reg = stage.tile([P, CH], F32, tag="st", name=f"xs_{p}_{q}")
                    nc.sync.dma_start(reg, x_d[p, :, q * CH : (q + 1) * CH])
                nc.scalar.activation(
                    reg, reg, AF.Identity, bias=magic_t[:], scale=inv_sx[:]
                )
                nc.vector.tensor_scalar(
                    qx[:, q * CH : (q + 1) * CH], reg, -MAGIC, None, op0=ALU.add
                )

            def quant_block(p):
                qx = qxp.tile([P, BLK], BF16, tag="qx", name=f"qx_{p}")
                for q in range(NXC):
                    quant_chunk(p, q, qx)
                return qx

            def wquant_chunk(e):
                nc.scalar.activation(
                    wf[e][:], wf[e][:], AF.Identity, bias=magic_t[:], scale=inv_sw[:]
                )
                nc.vector.tensor_scalar(
                    qwT[:, e * CH : (e + 1) * CH], wf[e][:], -MAGIC, None, op0=ALU.add
                )

            # Interleave: block 0+1 x-chunks woven between w chunks so the
            # first matmuls (and the first eviction) are never queue-blocked.
            qxb[0] = qxp.tile([P, BLK], BF16, tag="qx", name="qx_0")
            qxb[1] = qxp.tile([P, BLK], BF16, tag="qx", name="qx_1")
            for e in range(NWC):
                if e < NXC:
                    quant_chunk(0, e, qxb[0])
                wquant_chunk(e)
                if NWC - e <= NXC:
                    quant_chunk(1, e - (NWC - NXC), qxb[1])

            s_xw = scal.tile([P, 1], F32)
            nc.vector.tensor_tensor(out=s_xw[:], in0=s_x[:], in1=s_w[:], op=ALU.mult)
            s_b = scal.tile([P, 1], F32)
            bmax = scal.tile([P, 1], F32)
            nc.gpsimd.partition_all_reduce(
                bmax[:], am[:, nax + naw :], channels=P, reduce_op=bass_isa.ReduceOp.max
            )
            nc.vector.tensor_scalar(s_b[:], bmax[:], INV_QA, EPS, op0=ALU.mult, op1=ALU.max)
            inv_sb = scal.tile([P, 1], F32)
            nc.vector.reciprocal(inv_sb[:], s_b[:])
            by = scal.tile([P, n_jt], F32)
            nc.scalar.activation(by[:], bsh[:], AF.Identity, bias=magic_t[:], scale=inv_sb[:])
            bq = scal.tile([P, n_jt], F32)
            nc.vector.tensor_scalar(bq[:], by[:], -MAGIC, s_b[:], op0=ALU.add, op1=ALU.mult)

            # ---------------- Main: matmul + evict --------------------------
            for p in range(n_tb):
                if p + 2 < n_tb:
                    qxb[p + 2] = quant_block(p + 2)
                qx = qxb.pop(p)
                ps = [
                    mmps.tile([P, TB], F32, tag="mm", name=f"ps_{p}_{jt}")
                    for jt in range(n_jt)
                ]
                grps = [range(n_jt)] if p == 0 else [range(4), range(4, n_jt)]
                for g, jts in enumerate(grps):
                    for kt in range(n_kt):
                        for jt in jts:
                            nc.tensor.matmul(
                                ps[jt][:],
                                lhsT=qwT[:, kt * WPT + jt * P : kt * WPT + (jt + 1) * P],
                                rhs=qx[:, kt * TB : (kt + 1) * TB],
                                start=(kt == 0),
                                stop=(kt == n_kt - 1),
                            )
                    for jt in jts:
                        oc = opre[:, (p * n_jt + jt) * TB : (p * n_jt + jt + 1) * TB]
                        nc.scalar.activation(
                            oc, ps[jt][:], AF.Identity, bias=bq[:, jt : jt + 1], scale=s_xw[:]
                        )
                    lo = p * SLOT + jts[0] * TB
                    nc.vector.tensor_reduce(
                        omax[:, p * 2 + g : p * 2 + g + 1],
                        opre[:, lo : lo + len(jts) * TB],
                        axis=AX.X, op=ALU.max, apply_absolute_value=True,
                    )

            # ---------------- Tail: global out absmax -> requantize ---------
            om1 = scal.tile([P, 1], F32)
            nc.vector.tensor_reduce(om1[:], omax[:], axis=AX.X, op=ALU.max)
            omr = scal.tile([P, 1], F32)
            nc.gpsimd.partition_all_reduce(
                omr[:], om1[:], channels=P, reduce_op=bass_isa.ReduceOp.max
            )
            nc.sync.dma_start(cc2_in[:], omr[:1, :])
            nc.gpsimd.collective_compute(
                "AllReduce", ALU.max, replica_groups=groups,
                ins=[cc2_in[:]], outs=[cc2_out[:]],
            )
            go = scal.tile([P, 1], F32)
            nc.sync.dma_start(go[:1, :], cc2_out[:])
            bco = scal.tile([P, 1], F32)
            nc.gpsimd.partition_broadcast(bco[:], go[:1, :], channels=P)
            s_o = scal.tile([P, 1], F32)
            nc.vector.tensor_scalar(s_o[:], bco[:], INV_QA, EPS, op0=ALU.mult, op1=ALU.max)
            inv_so = scal.tile([P, 1], F32)
            nc.vector.reciprocal(inv_so[:], s_o[:])

            HS = SLOT // 2  # half-slot (4 jt) granularity for the tail
            for p in range(n_tb):
                for h in range(2):
                    src = opre[:, p * SLOT + h * HS : p * SLOT + (h + 1) * HS]
                    nc.scalar.activation(
                        src, src, AF.Identity, bias=magic_t[:], scale=inv_so[:]
                    )
                    res = typo.tile([P, HS], F32, tag="ores", name=f"res_{p}_{h}")
                    nc.vector.tensor_scalar(
                        res[:], src, -MAGIC, s_o[:], op0=ALU.add, op1=ALU.mult
                    )
                    nc.scalar.dma_start(
                        o_r[:, h * 4 : (h + 1) * 4, p * TB : (p + 1) * TB],
                        res[:].rearrange("p (jt t) -> p jt t", jt=4),
                    )

    nc.compile()
    return nc


def _pack_x(xa, a, bb, n_tb=8, TB=256, n_kt=32):
    # xa: [TA, K] token-slice for group a -> [n_tb, 128, n_kt*TB] packed,
    # k-rotated by a (matching w) and token-block-rotated by bb (excl-first).
    t = xa.reshape(n_tb, TB, n_kt, P).transpose(0, 3, 2, 1)  # [tb, p, kt, t]
    t = np.roll(t, -a * (n_kt // 2), axis=2)
    t = np.roll(t, -2 * bb, axis=0)
    return np.ascontiguousarray(t).reshape(n_tb, P, n_kt * TB)


def _pack_w(wb, a, n_kt=32):
    # wb: [JB, K] out-feature slice -> [128, n_kt*JB] packed, k-rotated by a.
    JB = wb.shape[0]
    t = wb.T.reshape(n_kt, P, JB).transpose(1, 0, 2)  # [p, kt, j]
    t = np.roll(t, -a * (n_kt // 2), axis=1)
    return np.ascontiguousarray(t).reshape(P, n_kt * JB)


def _run(nc, inputs, n_cores, T, K, J, trace=False):
    from concourse.bass_utils import run_bass_kernel_spmd

    TA, JB, TB = T // RT, J // RJ, 256
    n_tb = TA // TB
    x = np.ascontiguousarray(inputs["x"], dtype=np.float32)
    w = np.ascontiguousarray(inputs["weight"], dtype=np.float32)
    b = np.ascontiguousarray(inputs["b"], dtype=np.float32)
    in_maps = []
    for c in range(n_cores):
        a, bb = divmod(c, RJ)
        in_maps.append(
            {
                "x_p": _pack_x(x[a * TA : (a + 1) * TA], a, bb, n_tb, TB, K // P),
                "w_p": _pack_w(w[bb * JB : (bb + 1) * JB], a, K // P),
                "b_full": np.ascontiguousarray(b.reshape(P, J // P)),
                "b_shard": np.ascontiguousarray(
                    b[bb * JB : (bb + 1) * JB].reshape(JB // P, P).T
                ),
            }
        )
    res = run_bass_kernel_spmd(nc, in_maps, core_ids=list(range(n_cores)), trace=trace)
    out = np.empty((T, J), dtype=np.float32)
    for c in range(n_cores):
        a, bb = divmod(c, RJ)
        ot = res.results[c]["outT"]  # [JB, TA], token blocks rotated by bb
        ot = ot.reshape(JB, n_tb, TB)
        ot = np.roll(ot, 2 * bb, axis=1).reshape(JB, TA)
        out[a * TA : (a + 1) * TA, bb * JB : (bb + 1) * JB] = ot.T
    return out, res


_NC_CACHE = {}


def kernel(**inputs) -> np.ndarray:
    n_cores, T, K, J = 8, 4096, 4096, 4096
    key = (n_cores, T, K, J)
    if key not in _NC_CACHE:
        _NC_CACHE[key] = build(n_cores, T, K, J)
    out, _ = _run(_NC_CACHE[key], inputs, n_cores, T, K, J)
    return out


# revision 25
# speedup vs baseline: 1.5409x; 1.0414x over previous
"""Quantized Linear (8-bit act / 4-bit weight fake-quant) on 8 Trainium2 cores.

Math (per reference):
  xq = rne(x / s_x) * s_x          s_x = max(absmax(x)/127, 1e-8)
  wq = rne(w / s_w) * s_w          s_w = max(absmax(w)/7,   1e-8)
  bq = rne(b / s_b) * s_b          s_b = max(absmax(b)/127, 1e-8)
  out_pre = bq + xq @ wq.T
  out = rne(out_pre / s_o) * s_o   s_o = max(absmax(out_pre)/127, 1e-8)

Device strategy (2-way tokens x 4-way out_features, 8 cores):
  - Host packs per-core inputs k-major so the contraction dim lands on SBUF
    partitions with 8-32KB contiguous DMA lines: x -> [8 blk, 128, 8192]
    (blk-major, [kt,t] flat per partition), w -> [128, 32*1024] ([kt,j] flat).
    The PE does ONLY matmuls; no transposes anywhere.
  - Quantized integers Qx in [-127,127] / Qw in [-7,7] are exact in bf16 and
    accumulate exactly in fp32 PSUM; scales fold in afterwards:
    out_pre = (Qx@Qw)*(s_x*s_w) + bq.
  - Round-to-nearest-even via the fp32 magic constant (t + 1.5*2^23 then
    subtract). Quantization runs in [128,2048] chunks (ACT magic-add in
    place, DVE subtract to bf16) to amortize per-instruction overheads.
  - Global absmaxes via exclusive slices + one tiny AllReduce-max: each core
    reads 1/8 of x (its first two token blocks, host-rotated; bytes land in
    the opre buffer and are consumed from there) and 1/8 of w (a k-tile
    half; the host k-rotates BOTH x and w per core, so programs stay
    SPMD-identical and contraction order is irrelevant). DVE absmax reduces
    are issued in expected data-arrival order to avoid head-of-line blocks;
    W main loads issue from the (idle) PE queue so no ring stalls them.
  - Per 256-token block: 8 PSUM half-bank accumulators; block 0 runs one
    8-bank group (it chases the W DMA stream anyway), later blocks run two
    jt-groups of 4 so eviction overlaps the other group's matmul. Second
    AllReduce-max over out_pre, then requantize + store per half block.
"""

import sys

sys.path.insert(0, "/opt/trn_rl_repo")

import numpy as np

import concourse.bass as bass
import concourse.mybir as mybir
import concourse.tile as tile
from concourse import bacc, bass_isa

F32 = mybir.dt.float32
BF16 = mybir.dt.bfloat16
AF = mybir.ActivationFunctionType
ALU = mybir.AluOpType
AX = mybir.AxisListType

MAGIC = 12582912.0  # 1.5 * 2**23: fp32 add rounds to nearest-even integer
EPS = 1e-8
INV_QA = float(np.float32(1.0) / np.float32(127.0))
INV_QW = float(np.float32(1.0) / np.float32(7.0))

P = 128
RT, RJ = 2, 4  # token groups x out-feature groups


def build(n_cores=8, T=4096, K=4096, J=4096, TB=256):
    TA = T // RT  # 2048 tokens per core
    JB = J // RJ  # 1024 out features per core
    n_kt = K // P  # 32 k-tiles
    n_tb = TA // TB  # 8 token blocks
    n_jt = JB // P  # 8 j-tiles
    BLK = n_kt * TB  # 8192 floats per partition per x block
    SLOT = n_jt * TB  # 2048 floats per opre block slot
    WPT = JB  # w floats per partition per k-tile (1024)
    CH = 2048  # elementwise chunk size
    NWC = n_kt * WPT // CH  # 16 w chunks
    NXC = BLK // CH  # 4 x chunks per block

    nc = bacc.Bacc(
        "TRN2", target_bir_lowering=False, debug=False, num_devices=n_cores
    )

    x_d = nc.dram_tensor("x_p", [n_tb, P, BLK], F32, kind="ExternalInput")
    w_d = nc.dram_tensor("w_p", [P, n_kt * WPT], F32, kind="ExternalInput")
    b_d = nc.dram_tensor("b_full", [P, J // P], F32, kind="ExternalInput")
    bs_d = nc.dram_tensor("b_shard", [P, JB // P], F32, kind="ExternalInput")
    o_d = nc.dram_tensor("outT", [JB, TA], F32, kind="ExternalOutput")
    o_r = o_d.rearrange("(jt p) t -> p jt t", p=P)
    cc0_in = nc.dram_tensor("cc0_in", [1, 1], F32)
    cc0_out = nc.dram_tensor("cc0_out", [1, 1], F32)
    cc1_in = nc.dram_tensor("cc1_in", [1, 2], F32)
    cc1_out = nc.dram_tensor("cc1_out", [1, 2], F32)
    cc2_in = nc.dram_tensor("cc2_in", [1, 1], F32)
    cc2_out = nc.dram_tensor("cc2_out", [1, 1], F32)
    groups = [list(range(n_cores))]

    with tile.TileContext(nc) as tc:
        with (
            tc.tile_pool(name="const", bufs=1) as const,
            tc.tile_pool(name="scal", bufs=1) as scal,
            tc.tile_pool(name="qwp", bufs=1) as qwp,
            tc.tile_pool(name="qxp", bufs=2) as qxp,
            tc.tile_pool(name="big", bufs=1) as big,
            tc.tile_pool(name="stage", bufs=4) as stage,
            tc.tile_pool(name="typo", bufs=2) as typo,
            tc.tile_pool(name="mmps", bufs=8, space="PSUM") as mmps,
        ):
            bfull = scal.tile([P, J // P], F32)
            nc.gpsimd.dma_start(bfull[:], b_d[:, :])
            bsh = scal.tile([P, n_jt], F32)
            nc.gpsimd.dma_start(bsh[:], bs_d[:, :])
            # Warm-up collective: absorbs the one-time CC-ring setup (~10us
            # trigger delay) and core alignment while the exclusive-slice
            # DMAs stream, so the real absmax AllReduce starts instantly.
            nc.gpsimd.collective_compute(
                "AllReduce", ALU.max, replica_groups=groups,
                ins=[cc0_in[:]], outs=[cc0_out[:]],
            )
            magic_t = const.tile([P, 1], F32)
            nc.vector.memset(magic_t[:], MAGIC)
            omax = scal.tile([P, n_tb * 2], F32)
            nc.vector.memset(omax[:], 0.0)

            # ---------------- Phase 0: exclusive-slice absmaxes -------------
            nax = 2 * (BLK // 4096)  # 4 x reduce columns
            naw = (n_kt // 2) * WPT // CH  # 8 w reduce columns
            am = scal.tile([P, nax + naw + 1], F32)

            opre = big.tile([P, n_tb * SLOT], F32)
            # issue all exclusive-slice DMAs first (x in 2MiB halves so the
            # absmax reduces can start as early as possible) ...
            for i in range(2):
                for h in range(2):
                    nc.sync.dma_start(
                        opre[:, i * BLK + h * 4096 : i * BLK + (h + 1) * 4096],
                        x_d[i, :, h * 4096 : (h + 1) * 4096],
                    )
            wex = []
            for e in range(naw):
                t = stage.tile([P, CH], F32, tag="st", name=f"wex_{e}")
                nc.scalar.dma_start(t[:], w_d[:, e * CH : (e + 1) * CH])
                wex.append(t)
            # ... then DVE reduces in expected arrival order (w pieces land
            # every ~3.5us; x block i completes at ~30/45us).
            def wred(e):
                nc.vector.tensor_reduce(
                    am[:, nax + e : nax + e + 1], wex[e][:],
                    axis=AX.X, op=ALU.max, apply_absolute_value=True,
                )

            def xred(i, h):
                nc.vector.tensor_reduce(
                    am[:, i * 2 + h : i * 2 + h + 1],
                    opre[:, i * BLK + h * 4096 : i * BLK + (h + 1) * 4096],
                    axis=AX.X, op=ALU.max, apply_absolute_value=True,
                )

            # interleaved by expected arrival (x and w stream concurrently
            # on separate queues at roughly equal fabric share)
            wred(0)
            xred(0, 0)
            wred(1)
            wred(2)
            xred(0, 1)
            wred(3)
            wred(4)
            xred(1, 0)
            wred(5)
            wred(6)
            xred(1, 1)
            wred(7)
            nc.vector.tensor_reduce(
                am[:, nax + naw :], bfull[:], axis=AX.X, op=ALU.max,
                apply_absolute_value=True,
            )

            m2 = scal.tile([P, 2], F32)
            nc.vector.tensor_reduce(m2[:, 0:1], am[:, :nax], axis=AX.X, op=ALU.max)
            nc.vector.tensor_reduce(
                m2[:, 1:2], am[:, nax : nax + naw], axis=AX.X, op=ALU.max
            )
            g2 = scal.tile([P, 2], F32)
            nc.gpsimd.partition_all_reduce(
                g2[:], m2[:], channels=P, reduce_op=bass_isa.ReduceOp.max
            )
            nc.sync.dma_start(cc1_in[:], g2[:1, :])
            # First 4 W main chunks fit in free stage buffers: stream them
            # during the collective wait, when the DMA fabric is idle (after
            # the exclusive reads that gate the collective are done).
            wf = []
            for e in range(4):
                t = stage.tile([P, CH], F32, tag="st", name=f"wf_{e}")
                nc.sync.dma_start(t[:], w_d[:, e * CH : (e + 1) * CH])
                wf.append(t)
            nc.gpsimd.collective_compute(
                "AllReduce", ALU.max, replica_groups=groups,
                ins=[cc1_in[:]], outs=[cc1_out[:]],
            )
            gx = scal.tile([P, 2], F32)
            nc.sync.dma_start(gx[:1, :], cc1_out[:])
            # Stream the remaining W chunks as soon as the collective is done
            # (their stage buffers only free up once quantization consumes
            # the early chunks, so these must not block any earlier ring op).
            for e in range(4, NWC):
                t = stage.tile([P, CH], F32, tag="st", name=f"wf_{e}")
                nc.sync.dma_start(t[:], w_d[:, e * CH : (e + 1) * CH])
                wf.append(t)
            bc2 = scal.tile([P, 2], F32)
            nc.gpsimd.partition_broadcast(bc2[:], gx[:1, :], channels=P)

            s_x = scal.tile([P, 1], F32)
            s_w = scal.tile([P, 1], F32)
            nc.vector.tensor_scalar(s_x[:], bc2[:, 0:1], INV_QA, EPS, op0=ALU.mult, op1=ALU.max)
            inv_sx = scal.tile([P, 1], F32)
            nc.vector.reciprocal(inv_sx[:], s_x[:])
            nc.vector.tensor_scalar(s_w[:], bc2[:, 1:2], INV_QW, EPS, op0=ALU.mult, op1=ALU.max)
            inv_sw = scal.tile([P, 1], F32)
            nc.vector.reciprocal(inv_sw[:], s_w[:])

            # ---------------- Quantization helpers --------------------------
            qwT = qwp.tile([P, n_kt * WPT], BF16)
            qxb = {}

            def quant_chunk(p, q, qx):
                if p < 2:
                    reg = opre[:, p * BLK + q * CH : p * BLK + (q + 1) * CH]
                else:
                    reg = stage.tile([P, CH], F32, tag="st", name=f"xs_{p}_{q}")
                    nc.sync.dma_start(reg, x_d[p, :, q * CH : (q + 1) * CH])
                nc.scalar.activation(
                    reg, reg, AF.Identity, bias=magic_t[:], scale=inv_sx[:]
                )
                nc.vector.tensor_scalar(
                    qx[:, q * CH : (q + 1) * CH], reg, -MAGIC, None, op0=ALU.add
                )

            def quant_block(p):
                qx = qxp.tile([P, BLK], BF16, tag="qx", name=f"qx_{p}")
                for q in range(NXC):
                    quant_chunk(p, q, qx)
                return qx

            def wquant_chunk(e):
                nc.scalar.activation(
                    wf[e][:], wf[e][:], AF.Identity, bias=magic_t[:], scale=inv_sw[:]
                )
                nc.vector.tensor_scalar(
                    qwT[:, e * CH : (e + 1) * CH], wf[e][:], -MAGIC, None, op0=ALU.add
                )

            # Interleave: block 0+1 x-chunks woven between w chunks so the
            # first matmuls (and the first eviction) are never queue-blocked.
            qxb[0] = qxp.tile([P, BLK], BF16, tag="qx", name="qx_0")
            qxb[1] = qxp.tile([P, BLK], BF16, tag="qx", name="qx_1")
            for e in range(NWC):
                if e < NXC:
                    quant_chunk(0, e, qxb[0])
                wquant_chunk(e)
                if NWC - e <= NXC:
                    quant_chunk(1, e - (NWC - NXC), qxb[1])

            s_xw = scal.tile([P, 1], F32)
            nc.vector.tensor_tensor(out=s_xw[:], in0=s_x[:], in1=s_w[:], op=ALU.mult)
            s_b = scal.tile([P, 1], F32)
            bmax = scal.tile([P, 1], F32)
            nc.gpsimd.partition_all_reduce(
                bmax[:], am[:, nax + naw :], channels=P, reduce_op=bass_isa.ReduceOp.max
            )
            nc.vector.tensor_scalar(s_b[:], bmax[:], INV_QA, EPS, op0=ALU.mult, op1=ALU.max)
            inv_sb = scal.tile([P, 1], F32)
            nc.vector.reciprocal(inv_sb[:], s_b[:])
            by = scal.tile([P, n_jt], F32)
            nc.scalar.activation(by[:], bsh[:], AF.Identity, bias=magic_t[:], scale=inv_sb[:])
            bq = scal.tile([P, n_jt], F32)
            nc.vector.tensor_scalar(bq[:], by[:], -MAGIC, s_b[:], op0=ALU.add, op1=ALU.mult)

            # ---------------- Main: matmul + evict --------------------------
            for p in range(n_tb):
                if p + 2 < n_tb:
                    qxb[p + 2] = quant_block(p + 2)
                qx = qxb.pop(p)
                ps = [
                    mmps.tile([P, TB], F32, tag="mm", name=f"ps_{p}_{jt}")
                    for jt in range(n_jt)
                ]
                grps = [range(n_jt)] if p == 0 else [range(4), range(4, n_jt)]
                for g, jts in enumerate(grps):
                    for kt in range(n_kt):
                        for jt in jts:
                            nc.tensor.matmul(
                                ps[jt][:],
                                lhsT=qwT[:, kt * WPT + jt * P : kt * WPT + (jt + 1) * P],
                                rhs=qx[:, kt * TB : (kt + 1) * TB],
                                start=(kt == 0),
                                stop=(kt == n_kt - 1),
                            )
                    for jt in jts:
                        oc = opre[:, (p * n_jt + jt) * TB : (p * n_jt + jt + 1) * TB]
                        nc.scalar.activation(
                            oc, ps[jt][:], AF.Identity, bias=bq[:, jt : jt + 1], scale=s_xw[:]
                        )
                    lo = p * SLOT + jts[0] * TB
                    nc.vector.tensor_reduce(
                        omax[:, p * 2 + g : p * 2 + g + 1],
                        opre[:, lo : lo + len(jts) * TB],
                        axis=AX.X, op=ALU.max, apply_absolute_value=True,
                    )

            # ---------------- Tail: global out absmax -> requantize ---------
            om1 = scal.tile([P, 1], F32)
            nc.vector.tensor_reduce(om1[:], omax[:], axis=AX.X, op=ALU.max)
            omr = scal.tile([P, 1], F32)
            nc.gpsimd.partition_all_reduce(
                omr[:], om1[:], channels=P, reduce_op=bass_isa.ReduceOp.max
            )
            nc.sync.dma_start(cc2_in[:], omr[:1, :])
            nc.gpsimd.collective_compute(
                "AllReduce", ALU.max, replica_groups=groups,
                ins=[cc2_in[:]], outs=[cc2_out[:]],
            )
            go = scal.tile([P, 1], F32)
            nc.sync.dma_start(go[:1, :], cc2_out[:])
            bco = scal.tile([P, 1], F32)
            nc.gpsimd.partition_broadcast(bco[:], go[:1, :], channels=P)
            s_o = scal.tile([P, 1], F32)
            nc.vector.tensor_scalar(s_o[:], bco[:], INV_QA, EPS, op0=ALU.mult, op1=ALU.max)
            inv_so = scal.tile([P, 1], F32)
            nc.vector.reciprocal(inv_so[:], s_o[:])

            HS = SLOT // 2  # half-slot (4 jt) granularity for the tail
            for p in range(n_tb):
                for h in range(2):
                    src = opre[:, p * SLOT + h * HS : p * SLOT + (h + 1) * HS]
                    nc.scalar.activation(
                        src, src, AF.Identity, bias=magic_t[:], scale=inv_so[:]
                    )
                    res = typo.tile([P, HS], F32, tag="ores", name=f"res_{p}_{h}")
                    nc.vector.tensor_scalar(
                        res[:], src, -MAGIC, s_o[:], op0=ALU.add, op1=ALU.mult
                    )
                    nc.scalar.dma_start(
                        o_r[:, h * 4 : (h + 1) * 4, p * TB : (p + 1) * TB],
                        res[:].rearrange("p (jt t) -> p jt t", jt=4),
                    )

    nc.compile()
    return nc


def _pack_x(xa, a, bb, n_tb=8, TB=256, n_kt=32):
    # xa: [TA, K] token-slice for group a -> [n_tb, 128, n_kt*TB] packed,
    # k-rotated by a (matching w) and token-block-rotated by bb (excl-first).
    t = xa.reshape(n_tb, TB, n_kt, P).transpose(0, 3, 2, 1)  # [tb, p, kt, t]
    t = np.roll(t, -a * (n_kt // 2), axis=2)
    t = np.roll(t, -2 * bb, axis=0)
    return np.ascontiguousarray(t).reshape(n_tb, P, n_kt * TB)


def _pack_w(wb, a, n_kt=32):
    # wb: [JB, K] out-feature slice -> [128, n_kt*JB] packed, k-rotated by a.
    JB = wb.shape[0]
    t = wb.T.reshape(n_kt, P, JB).transpose(1, 0, 2)  # [p, kt, j]
    t = np.roll(t, -a * (n_kt // 2), axis=1)
    return np.ascontiguousarray(t).reshape(P, n_kt * JB)


def _run(nc, inputs, n_cores, T, K, J, trace=False):
    from concourse.bass_utils import run_bass_kernel_spmd

    TA, JB, TB = T // RT, J // RJ, 256
    n_tb = TA // TB
    x = np.ascontiguousarray(inputs["x"], dtype=np.float32)
    w = np.ascontiguousarray(inputs["weight"], dtype=np.float32)
    b = np.ascontiguousarray(inputs["b"], dtype=np.float32)
    in_maps = []
    for c in range(n_cores):
        a, bb = divmod(c, RJ)
        in_maps.append(
            {
                "x_p": _pack_x(x[a * TA : (a + 1) * TA], a, bb, n_tb, TB, K // P),
                "w_p": _pack_w(w[bb * JB : (bb + 1) * JB], a, K // P),
                "b_full": np.ascontiguousarray(b.reshape(P, J // P)),
                "b_shard": np.ascontiguousarray(
                    b[bb * JB : (bb + 1) * JB].reshape(JB // P, P).T
                ),
            }
        )
    res = run_bass_kernel_spmd(nc, in_maps, core_ids=list(range(n_cores)), trace=trace)
    out = np.empty((T, J), dtype=np.float32)
    for c in range(n_cores):
        a, bb = divmod(c, RJ)
        ot = res.results[c]["outT"]  # [JB, TA], token blocks rotated by bb
        ot = ot.reshape(JB, n_tb, TB)
        ot = np.roll(ot, 2 * bb, axis=1).reshape(JB, TA)
        out[a * TA : (a + 1) * TA, bb * JB : (bb + 1) * JB] = ot.T
    return out, res


_NC_CACHE = {}


def kernel(**inputs) -> np.ndarray:
    n_cores, T, K, J = 8, 4096, 4096, 4096
    key = (n_cores, T, K, J)
    if key not in _NC_CACHE:
        _NC_CACHE[key] = build(n_cores, T, K, J)
    out, _ = _run(_NC_CACHE[key], inputs, n_cores, T, K, J)
    return out


# revision 26
# speedup vs baseline: 1.6354x; 1.0614x over previous
"""Quantized Linear (8-bit act / 4-bit weight fake-quant) on 8 Trainium2 cores.

Math (per reference):
  xq = rne(x / s_x) * s_x          s_x = max(absmax(x)/127, 1e-8)
  wq = rne(w / s_w) * s_w          s_w = max(absmax(w)/7,   1e-8)
  bq = rne(b / s_b) * s_b          s_b = max(absmax(b)/127, 1e-8)
  out_pre = bq + xq @ wq.T
  out = rne(out_pre / s_o) * s_o   s_o = max(absmax(out_pre)/127, 1e-8)

Device strategy (2-way tokens x 4-way out_features, 8 cores):
  - Host packs per-core inputs k-major so the contraction dim lands on SBUF
    partitions with 8-32KB contiguous DMA lines: x -> [8 blk, 128, 8192]
    (blk-major, [kt,t] flat per partition), w -> [128, 32*1024] ([kt,j] flat).
    The PE does ONLY matmuls; no transposes anywhere.
  - Quantized integers Qx in [-127,127] / Qw in [-7,7] are exact in bf16 and
    accumulate exactly in fp32 PSUM; scales fold in afterwards:
    out_pre = (Qx@Qw)*(s_x*s_w) + bq.
  - Round-to-nearest-even via the fp32 magic constant (t + 1.5*2^23 then
    subtract). Quantization runs in [128,2048] chunks (ACT magic-add in
    place, DVE subtract to bf16) to amortize per-instruction overheads.
  - Global absmaxes via exclusive slices + one tiny AllReduce-max: each core
    reads 1/8 of x (its first two token blocks, host-rotated; bytes land in
    the opre buffer and are consumed from there) and 1/8 of w (a k-tile
    half; the host k-rotates BOTH x and w per core, so programs stay
    SPMD-identical and contraction order is irrelevant). DVE absmax reduces
    are issued in expected data-arrival order to avoid head-of-line blocks;
    W main loads issue from the (idle) PE queue so no ring stalls them.
  - Per 256-token block: 8 PSUM half-bank accumulators; block 0 runs one
    8-bank group (it chases the W DMA stream anyway), later blocks run two
    jt-groups of 4 so eviction overlaps the other group's matmul. Second
    AllReduce-max over out_pre, then requantize + store per half block.
"""

import sys

sys.path.insert(0, "/opt/trn_rl_repo")

import numpy as np

import concourse.bass as bass
import concourse.mybir as mybir
import concourse.tile as tile
from concourse import bacc, bass_isa

F32 = mybir.dt.float32
BF16 = mybir.dt.bfloat16
AF = mybir.ActivationFunctionType
ALU = mybir.AluOpType
AX = mybir.AxisListType

MAGIC = 12582912.0  # 1.5 * 2**23: fp32 add rounds to nearest-even integer
EPS = 1e-8
INV_QA = float(np.float32(1.0) / np.float32(127.0))
INV_QW = float(np.float32(1.0) / np.float32(7.0))

P = 128
RT, RJ = 2, 4  # token groups x out-feature groups


def build(n_cores=8, T=4096, K=4096, J=4096, TB=256):
    TA = T // RT  # 2048 tokens per core
    JB = J // RJ  # 1024 out features per core
    n_kt = K // P  # 32 k-tiles
    n_tb = TA // TB  # 8 token blocks
    n_jt = JB // P  # 8 j-tiles
    BLK = n_kt * TB  # 8192 floats per partition per x block
    SLOT = n_jt * TB  # 2048 floats per opre block slot
    WPT = JB  # w floats per partition per k-tile (1024)
    CH = 2048  # elementwise chunk size
    NWC = n_kt * WPT // CH  # 16 w chunks
    NXC = BLK // CH  # 4 x chunks per block

    nc = bacc.Bacc(
        "TRN2", target_bir_lowering=False, debug=False, num_devices=n_cores
    )

    x_d = nc.dram_tensor("x_p", [n_tb, P, BLK], F32, kind="ExternalInput")
    w_d = nc.dram_tensor("w_p", [P, n_kt * WPT], F32, kind="ExternalInput")
    b_d = nc.dram_tensor("b_full", [P, J // P], F32, kind="ExternalInput")
    bs_d = nc.dram_tensor("b_shard", [1, JB], F32, kind="ExternalInput")
    o_d = nc.dram_tensor("out_s", [TA, JB], F32, kind="ExternalOutput")
    cc0_in = nc.dram_tensor("cc0_in", [1, 1], F32)
    cc0_out = nc.dram_tensor("cc0_out", [1, 1], F32)
    cc1_in = nc.dram_tensor("cc1_in", [1, 2], F32)
    cc1_out = nc.dram_tensor("cc1_out", [1, 2], F32)
    cc2_in = nc.dram_tensor("cc2_in", [1, 1], F32)
    cc2_out = nc.dram_tensor("cc2_out", [1, 1], F32)
    groups = [list(range(n_cores))]

    with tile.TileContext(nc) as tc:
        with (
            tc.tile_pool(name="const", bufs=1) as const,
            tc.tile_pool(name="scal", bufs=1) as scal,
            tc.tile_pool(name="qwp", bufs=1) as qwp,
            tc.tile_pool(name="qxp", bufs=2) as qxp,
            tc.tile_pool(name="big", bufs=1) as big,
            tc.tile_pool(name="stage", bufs=3) as stage,
            tc.tile_pool(name="typo", bufs=3) as typo,
            tc.tile_pool(name="mmps", bufs=8, space="PSUM") as mmps,
        ):
            bfull = scal.tile([P, J // P], F32)
            nc.gpsimd.dma_start(bfull[:], b_d[:, :])
            bsr = scal.tile([1, JB], F32)
            nc.gpsimd.dma_start(bsr[:], bs_d[:, :])
            # Warm-up collective: absorbs the one-time CC-ring setup (~10us
            # trigger delay) and core alignment while the exclusive-slice
            # DMAs stream, so the real absmax AllReduce starts instantly.
            nc.gpsimd.collective_compute(
                "AllReduce", ALU.max, replica_groups=groups,
                ins=[cc0_in[:]], outs=[cc0_out[:]],
            )
            magic_t = const.tile([P, 1], F32)
            nc.vector.memset(magic_t[:], MAGIC)
            omax = scal.tile([P, n_tb], F32)
            nc.vector.memset(omax[:], 0.0)

            # ---------------- Phase 0: exclusive-slice absmaxes -------------
            nax = 2 * (BLK // 4096)  # 4 x reduce columns
            naw = (n_kt // 2) * WPT // CH  # 8 w reduce columns
            am = scal.tile([P, nax + naw + 1], F32)

            opre = big.tile([P, n_tb * SLOT], F32)
            # issue all exclusive-slice DMAs first (x in 2MiB halves so the
            # absmax reduces can start as early as possible) ...
            for i in range(2):
                for h in range(2):
                    nc.sync.dma_start(
                        opre[:, i * BLK + h * 4096 : i * BLK + (h + 1) * 4096],
                        x_d[i, :, h * 4096 : (h + 1) * 4096],
                    )
            wex = []
            for e in range(naw):
                t = stage.tile([P, CH], F32, tag="st", name=f"wex_{e}")
                nc.scalar.dma_start(t[:], w_d[:, e * CH : (e + 1) * CH])
                wex.append(t)
            # ... then DVE reduces in expected arrival order (w pieces land
            # every ~3.5us; x block i completes at ~30/45us).
            def wred(e):
                nc.vector.tensor_reduce(
                    am[:, nax + e : nax + e + 1], wex[e][:],
                    axis=AX.X, op=ALU.max, apply_absolute_value=True,
                )

            def xred(i, h):
                nc.vector.tensor_reduce(
                    am[:, i * 2 + h : i * 2 + h + 1],
                    opre[:, i * BLK + h * 4096 : i * BLK + (h + 1) * 4096],
                    axis=AX.X, op=ALU.max, apply_absolute_value=True,
                )

            # interleaved by expected arrival (x and w stream concurrently
            # on separate queues at roughly equal fabric share)
            wred(0)
            xred(0, 0)
            wred(1)
            wred(2)
            xred(0, 1)
            wred(3)
            wred(4)
            xred(1, 0)
            wred(5)
            wred(6)
            xred(1, 1)
            wred(7)
            nc.vector.tensor_reduce(
                am[:, nax + naw :], bfull[:], axis=AX.X, op=ALU.max,
                apply_absolute_value=True,
            )

            m2 = scal.tile([P, 2], F32)
            nc.vector.tensor_reduce(m2[:, 0:1], am[:, :nax], axis=AX.X, op=ALU.max)
            nc.vector.tensor_reduce(
                m2[:, 1:2], am[:, nax : nax + naw], axis=AX.X, op=ALU.max
            )
            g2 = scal.tile([P, 2], F32)
            nc.gpsimd.partition_all_reduce(
                g2[:], m2[:], channels=P, reduce_op=bass_isa.ReduceOp.max
            )
            nc.sync.dma_start(cc1_in[:], g2[:1, :])
            # First 4 W main chunks fit in free stage buffers: stream them
            # during the collective wait, when the DMA fabric is idle (after
            # the exclusive reads that gate the collective are done).
            wf = []
            for e in range(3):
                t = stage.tile([P, CH], F32, tag="st", name=f"wf_{e}")
                nc.sync.dma_start(t[:], w_d[:, e * CH : (e + 1) * CH])
                wf.append(t)
            nc.gpsimd.collective_compute(
                "AllReduce", ALU.max, replica_groups=groups,
                ins=[cc1_in[:]], outs=[cc1_out[:]],
            )
            gx = scal.tile([P, 2], F32)
            nc.sync.dma_start(gx[:1, :], cc1_out[:])
            # Stream the remaining W chunks as soon as the collective is done
            # (their stage buffers only free up once quantization consumes
            # the early chunks, so these must not block any earlier ring op).
            for e in range(3, NWC):
                t = stage.tile([P, CH], F32, tag="st", name=f"wf_{e}")
                nc.sync.dma_start(t[:], w_d[:, e * CH : (e + 1) * CH])
                wf.append(t)
            bc2 = scal.tile([P, 2], F32)
            nc.gpsimd.partition_broadcast(bc2[:], gx[:1, :], channels=P)

            s_x = scal.tile([P, 1], F32)
            s_w = scal.tile([P, 1], F32)
            nc.vector.tensor_scalar(s_x[:], bc2[:, 0:1], INV_QA, EPS, op0=ALU.mult, op1=ALU.max)
            inv_sx = scal.tile([P, 1], F32)
            nc.vector.reciprocal(inv_sx[:], s_x[:])
            nc.vector.tensor_scalar(s_w[:], bc2[:, 1:2], INV_QW, EPS, op0=ALU.mult, op1=ALU.max)
            inv_sw = scal.tile([P, 1], F32)
            nc.vector.reciprocal(inv_sw[:], s_w[:])

            # ---------------- Quantization helpers --------------------------
            qwT = qwp.tile([P, n_kt * WPT], BF16)
            qxb = {}

            def quant_chunk(p, q, qx):
                if p < 2:
                    reg = opre[:, p * BLK + q * CH : p * BLK + (q + 1) * CH]
                else:
                    reg = stage.tile([P, CH], F32, tag="st", name=f"xs_{p}_{q}")
                    nc.sync.dma_start(reg, x_d[p, :, q * CH : (q + 1) * CH])
                nc.scalar.activation(
                    reg, reg, AF.Identity, bias=magic_t[:], scale=inv_sx[:]
                )
                nc.vector.tensor_scalar(
                    qx[:, q * CH : (q + 1) * CH], reg, -MAGIC, None, op0=ALU.add
                )

            def quant_block(p):
                qx = qxp.tile([P, BLK], BF16, tag="qx", name=f"qx_{p}")
                for q in range(NXC):
                    quant_chunk(p, q, qx)
                return qx

            def wquant_chunk(e):
                nc.scalar.activation(
                    wf[e][:], wf[e][:], AF.Identity, bias=magic_t[:], scale=inv_sw[:]
                )
                nc.vector.tensor_scalar(
                    qwT[:, e * CH : (e + 1) * CH], wf[e][:], -MAGIC, None, op0=ALU.add
                )

            # Interleave: block 0+1 x-chunks woven between w chunks so the
            # first matmuls (and the first eviction) are never queue-blocked.
            qxb[0] = qxp.tile([P, BLK], BF16, tag="qx", name="qx_0")
            qxb[1] = qxp.tile([P, BLK], BF16, tag="qx", name="qx_1")
            for e in range(NWC):
                if e < NXC:
                    quant_chunk(0, e, qxb[0])
                wquant_chunk(e)
                if NWC - e <= NXC:
                    quant_chunk(1, e - (NWC - NXC), qxb[1])

            s_xw = scal.tile([P, 1], F32)
            nc.vector.tensor_tensor(out=s_xw[:], in0=s_x[:], in1=s_w[:], op=ALU.mult)
            s_b = scal.tile([P, 1], F32)
            bmax = scal.tile([P, 1], F32)
            nc.gpsimd.partition_all_reduce(
                bmax[:], am[:, nax + naw :], channels=P, reduce_op=bass_isa.ReduceOp.max
            )
            nc.vector.tensor_scalar(s_b[:], bmax[:], INV_QA, EPS, op0=ALU.mult, op1=ALU.max)
            inv_sb = scal.tile([P, 1], F32)
            nc.vector.reciprocal(inv_sb[:], s_b[:])
            nc.scalar.activation(
                bsr[:], bsr[:], AF.Identity, bias=magic_t[:1, :], scale=inv_sb[:1, :]
            )
            nc.vector.tensor_scalar(
                bsr[:], bsr[:], -MAGIC, s_b[:1, :], op0=ALU.add, op1=ALU.mult
            )
            bq_b = scal.tile([P, JB], F32)
            nc.gpsimd.partition_broadcast(bq_b[:], bsr[:1, :], channels=P)

            # ---------------- Main: matmul + evict --------------------------
            JH = WPT // 2  # 512 moving j columns per matmul

            def evict(p, th, jh, ps):
                oc = opre[:, p * SLOT + th * WPT + jh * JH : p * SLOT + th * WPT + (jh + 1) * JH]
                nc.vector.scalar_tensor_tensor(
                    oc, ps[:], s_xw[:], bq_b[:, jh * JH : (jh + 1) * JH],
                    op0=ALU.mult, op1=ALU.add,
                )

            for p in range(n_tb):
                if p + 2 < n_tb:
                    qxb[p + 2] = quant_block(p + 2)
                qx = qxb.pop(p)
                ps = [
                    mmps.tile([P, JH], F32, tag="mm", name=f"ps_{p}_{i}")
                    for i in range(4)
                ]
                if p == 0:
                    # block 0 chases the W DMA stream: consume each k-tile
                    # once (all 4 accumulators) so the pace matches arrivals
                    for kt in range(n_kt):
                        for th in range(2):
                            for jh in range(2):
                                nc.tensor.matmul(
                                    ps[th * 2 + jh][:],
                                    lhsT=qx[:, kt * TB + th * P : kt * TB + (th + 1) * P],
                                    rhs=qwT[:, kt * WPT + jh * JH : kt * WPT + (jh + 1) * JH],
                                    start=(kt == 0),
                                    stop=(kt == n_kt - 1),
                                )
                    for th in range(2):
                        for jh in range(2):
                            evict(p, th, jh, ps[th * 2 + jh])
                else:
                    # two th-groups of 2 banks: group 0's eviction overlaps
                    # group 1's matmuls
                    for th in range(2):
                        for kt in range(n_kt):
                            for jh in range(2):
                                nc.tensor.matmul(
                                    ps[th * 2 + jh][:],
                                    lhsT=qx[:, kt * TB + th * P : kt * TB + (th + 1) * P],
                                    rhs=qwT[:, kt * WPT + jh * JH : kt * WPT + (jh + 1) * JH],
                                    start=(kt == 0),
                                    stop=(kt == n_kt - 1),
                                )
                        for jh in range(2):
                            evict(p, th, jh, ps[th * 2 + jh])
                nc.vector.tensor_reduce(
                    omax[:, p : p + 1], opre[:, p * SLOT : (p + 1) * SLOT],
                    axis=AX.X, op=ALU.max, apply_absolute_value=True,
                )

            # ---------------- Tail: global out absmax -> requantize ---------
            om1 = scal.tile([P, 1], F32)
            nc.vector.tensor_reduce(om1[:], omax[:], axis=AX.X, op=ALU.max)
            omr = scal.tile([P, 1], F32)
            nc.gpsimd.partition_all_reduce(
                omr[:], om1[:], channels=P, reduce_op=bass_isa.ReduceOp.max
            )
            nc.sync.dma_start(cc2_in[:], omr[:1, :])
            nc.gpsimd.collective_compute(
                "AllReduce", ALU.max, replica_groups=groups,
                ins=[cc2_in[:]], outs=[cc2_out[:]],
            )
            go = scal.tile([P, 1], F32)
            nc.sync.dma_start(go[:1, :], cc2_out[:])
            bco = scal.tile([P, 1], F32)
            nc.gpsimd.partition_broadcast(bco[:], go[:1, :], channels=P)
            s_o = scal.tile([P, 1], F32)
            nc.vector.tensor_scalar(s_o[:], bco[:], INV_QA, EPS, op0=ALU.mult, op1=ALU.max)
            inv_so = scal.tile([P, 1], F32)
            nc.vector.reciprocal(inv_so[:], s_o[:])

            for p in range(n_tb):
                for th in range(2):
                    src = opre[:, p * SLOT + th * WPT : p * SLOT + (th + 1) * WPT]
                    nc.scalar.activation(
                        src, src, AF.Identity, bias=magic_t[:], scale=inv_so[:]
                    )
                    res = typo.tile([P, WPT], F32, tag="ores", name=f"res_{p}_{th}")
                    nc.vector.tensor_scalar(
                        res[:], src, -MAGIC, s_o[:], op0=ALU.add, op1=ALU.mult
                    )
                    nc.sync.dma_start(
                        o_d[p * TB + th * P : p * TB + (th + 1) * P, :], res[:]
                    )

    nc.compile()
    return nc


def _pack_x(xa, a, bb, n_tb=8, TB=256, n_kt=32):
    # xa: [TA, K] token-slice for group a -> [n_tb, 128, n_kt*TB] packed,
    # k-rotated by a (matching w) and token-block-rotated by bb (excl-first).
    t = xa.reshape(n_tb, TB, n_kt, P).transpose(0, 3, 2, 1)  # [tb, p, kt, t]
    t = np.roll(t, -a * (n_kt // 2), axis=2)
    t = np.roll(t, -2 * bb, axis=0)
    return np.ascontiguousarray(t).reshape(n_tb, P, n_kt * TB)


def _pack_w(wb, a, n_kt=32):
    # wb: [JB, K] out-feature slice -> [128, n_kt*JB] packed, k-rotated by a.
    JB = wb.shape[0]
    t = wb.T.reshape(n_kt, P, JB).transpose(1, 0, 2)  # [p, kt, j]
    t = np.roll(t, -a * (n_kt // 2), axis=1)
    return np.ascontiguousarray(t).reshape(P, n_kt * JB)


def _run(nc, inputs, n_cores, T, K, J, trace=False):
    from concourse.bass_utils import run_bass_kernel_spmd

    TA, JB, TB = T // RT, J // RJ, 256
    n_tb = TA // TB
    x = np.ascontiguousarray(inputs["x"], dtype=np.float32)
    w = np.ascontiguousarray(inputs["weight"], dtype=np.float32)
    b = np.ascontiguousarray(inputs["b"], dtype=np.float32)
    in_maps = []
    for c in range(n_cores):
        a, bb = divmod(c, RJ)
        in_maps.append(
            {
                "x_p": _pack_x(x[a * TA : (a + 1) * TA], a, bb, n_tb, TB, K // P),
                "w_p": _pack_w(w[bb * JB : (bb + 1) * JB], a, K // P),
                "b_full": np.ascontiguousarray(b.reshape(P, J // P)),
                "b_shard": np.ascontiguousarray(b[bb * JB : (bb + 1) * JB].reshape(1, JB)),
            }
        )
    res = run_bass_kernel_spmd(nc, in_maps, core_ids=list(range(n_cores)), trace=trace)
    out = np.empty((T, J), dtype=np.float32)
    for c in range(n_cores):
        a, bb = divmod(c, RJ)
        ot = res.results[c]["out_s"]  # [TA, JB], token blocks rotated by bb
        ot = ot.reshape(n_tb, TB, JB)
        ot = np.roll(ot, 2 * bb, axis=0).reshape(TA, JB)
        out[a * TA : (a + 1) * TA, bb * JB : (bb + 1) * JB] = ot
    return out, res


_NC_CACHE = {}


def kernel(**inputs) -> np.ndarray:
    n_cores, T, K, J = 8, 4096, 4096, 4096
    key = (n_cores, T, K, J)
    if key not in _NC_CACHE:
        _NC_CACHE[key] = build(n_cores, T, K, J)
    out, _ = _run(_NC_CACHE[key], inputs, n_cores, T, K, J)
    return out
